# revision 7
# baseline (speedup 1.0000x reference)
"""Trainium2 Bass kernel for the DiT block (B=4, N=1024, HID=1024, NH=16, MLP=4096).

Sharding: 8 cores = 4 batches x 2 sequence halves. Each core computes the
full block for its 512 tokens (feature-major layout [hidden, tokens]); K/V
for the other half of its batch are recomputed locally (~14% extra FLOPs,
zero collectives). Host gathers/concats the per-core [1024, 512] outputs.

Matmuls run in float32r (TF32-like, 1 cycle/row for moving dim >= 256).
Weight normalization and the tiny conditioning matmul (0.014% of FLOPs)
are folded on the host in float64.
"""

import math
import os

import numpy as np

B, N, HID, NH = 4, 1024, 1024, 16
DH = HID // NH
MLP = 4 * HID
P = 128
KC = HID // P          # 8 hidden chunks
MC = MLP // P          # 32 mlp chunks
TO = N // 2            # 512 own tokens per core
NCORES = 8

_TRACE = os.environ.get("DIT_TRACE", "0") == "1"
_DBG = os.environ.get("DIT_DEBUG_OUTPUTS", "0") == "1"
_MMDT_NAME = os.environ.get("DIT_MMDT", "f32r")  # f32r | bf16


def _normalize_w(w):
    w = np.asarray(w, np.float64)
    norm = np.linalg.norm(w, axis=1, keepdims=True)
    alpha = math.sqrt(w.shape[1])
    wn = w / (norm * alpha + 1e-4)
    wn = wn / math.sqrt(w.shape[0])
    return wn


def _host_prep(x, c, w_cond, w_qkv, w_out, w_mlp1, w_mlp2, attn_gain, mlp_gain):
    """Float64 host-side folding: weight norms, conditioning, scalars."""
    wn_qkv = _normalize_w(w_qkv)
    wn_out = _normalize_w(w_out)
    wn_mlp1 = _normalize_w(w_mlp1)
    wn_mlp2 = _normalize_w(w_mlp2)
    wn_cond = _normalize_w(w_cond)

    c64 = np.asarray(c, np.float64)
    silu = c64 / (1.0 + np.exp(-c64))
    cc = (silu / 0.596) @ wn_cond              # [B, 2H]
    gain, shift = cc[:, :HID], cc[:, HID:]
    g1 = 1.0 + gain

    ag_c = float(np.exp(np.float64(attn_gain)) * 0.3 / math.sqrt(0.58))
    mg_c = float(np.exp(np.float64(mlp_gain)) * 0.3 / math.sqrt(0.58))
    c1 = float(0.7 / math.sqrt(0.58))
    return (wn_qkv, wn_out, wn_mlp1, wn_mlp2, g1, shift, ag_c, mg_c, c1)


def _build_nc(ag_c, mg_c, c1):
    import concourse.bass as bass
    import concourse.tile as tile
    from concourse import bacc, mybir

    MMDT = mybir.dt.float32r if _MMDT_NAME == "f32r" else mybir.dt.bfloat16
    BF16 = mybir.dt.bfloat16
    F32 = mybir.dt.float32
    AF = mybir.ActivationFunctionType
    ALU = mybir.AluOpType

    nc = bacc.Bacc()

    # ---- dram I/O ----
    x_own_d = nc.dram_tensor("x_own", [HID, TO], F32, kind="ExternalInput")
    x_oth_d = nc.dram_tensor("x_oth", [HID, TO], F32, kind="ExternalInput")
    gs_d = nc.dram_tensor("gs", [P, KC, 2], F32, kind="ExternalInput")
    wqkv_d = nc.dram_tensor("w_qkv", [HID, 3 * HID], MMDT, kind="ExternalInput")
    wout_d = nc.dram_tensor("w_out", [HID, HID], MMDT, kind="ExternalInput")
    w1_d = nc.dram_tensor("w_mlp1", [HID, MLP], MMDT, kind="ExternalInput")
    w2_d = nc.dram_tensor("w_mlp2", [MLP, HID], MMDT, kind="ExternalInput")
    ones_d = nc.dram_tensor("ones_c", [P, P], BF16, kind="ExternalInput")
    bd_d = nc.dram_tensor("bd_c", [P, P], BF16, kind="ExternalInput")
    y_d = nc.dram_tensor("y", [HID, TO], F32, kind="ExternalOutput")

    dbg = {}
    if _DBG:
        for nm, shp in [("xc", [HID, N]), ("q", [HID, TO]), ("k", [HID, N]),
                        ("ao", [HID, TO]), ("x1", [HID, TO]), ("hmid", [MLP, TO]),
                        ("vtm", [N, HID])]:
            dbg[nm] = nc.dram_tensor("dbg_" + nm, shp, F32, kind="ExternalOutput")

    xo_r = x_own_d.rearrange("(kc ki) t -> ki kc t", ki=P)
    xt_r = x_oth_d.rearrange("(kc ki) t -> ki kc t", ki=P)
    wqkv_r = wqkv_d.rearrange("(kc ki) m -> ki kc m", ki=P)
    wout_r = wout_d.rearrange("(kc ki) m -> ki kc m", ki=P)
    w1_r = w1_d.rearrange("(kc ki) m -> ki kc m", ki=P)
    w2_r = w2_d.rearrange("(kc ki) m -> ki kc m", ki=P)
    y_r = y_d.rearrange("(kc ki) t -> ki kc t", ki=P)

    dma = nc.gpsimd.dma_start

    with tile.TileContext(nc) as tc:
        with tc.tile_pool(name="pp", bufs=1) as pp, \
             tc.tile_pool(name="pdbg", bufs=2) as pdbg, \
             tc.tile_pool(name="ps_acc", bufs=3, space="PSUM") as ps_acc, \
             tc.tile_pool(name="ps_norm", bufs=2, space="PSUM") as ps_norm, \
             tc.tile_pool(name="ps_long", bufs=2, space="PSUM") as ps_long, \
             tc.tile_pool(name="dram", bufs=1, space="DRAM") as drp:

            # ---- persistent small constants ----
            ones_c = pp.tile([P, P], BF16, tag="ones")
            dma(ones_c[:], ones_d[:, :])
            bd_c = pp.tile([P, P], BF16, tag="bd")
            dma(bd_c[:], bd_d[:, :])
            gs_sb = pp.tile([P, KC, 2], F32, tag="gs")
            dma(gs_sb[:], gs_d[:, :, :])
            eps4 = pp.tile([P, 1], F32, tag="eps4")
            nc.vector.memset(eps4[:], 1e-4)
            epsq = pp.tile([P, 1], F32, tag="epsq")
            nc.vector.memset(epsq[:], 1e-10)
            epsm = pp.tile([P, 1], F32, tag="epsm")
            nc.vector.memset(epsm[:], 1e-4 * 0.596 * 0.596)
            onecol = pp.tile([P, 1], F32, tag="onecol")
            nc.vector.memset(onecol[:], 1.0)

            x_own = pp.tile([P, KC, TO], F32, tag="xown")
            dma(x_own[:], xo_r)

            def pixel_scale(pool, psum, mean_div, eps_tile, tag):
                """scale = 1/sqrt(psum/mean_div + eps) as [P, TO] fp32."""
                s = pool.tile([P, TO], F32, tag=tag)
                nc.scalar.activation(s[:], psum[:], AF.Sqrt,
                                     scale=1.0 / mean_div, bias=eps_tile[:])
                nc.vector.reciprocal(s[:], s[:])
                return s

            def dbg_dump(name, src, nchunks, cols):
                """Stream src [P, nchunks, cols] (any dtype) to dbg[name]."""
                if not _DBG:
                    return
                dst = dbg[name].rearrange("(kc ki) t -> ki kc t", ki=P)
                for ch in range(nchunks):
                    t = pdbg.tile([P, cols], F32, tag="dbgt")
                    nc.vector.tensor_copy(t[:], src[:, ch, :])
                    dma(dst[:, ch, :], t[:])

            with tc.tile_pool(name="p_ao", bufs=1) as p_ao:
                with tc.tile_pool(name="p_qkv", bufs=1) as p_qkv:

                    # ============ phase 1+2: x_cond, QKV ============
                    with tc.tile_pool(name="p12", bufs=1) as p12, \
                         tc.tile_pool(name="p12w", bufs=2) as p12w, \
                         tc.tile_pool(name="p12v", bufs=1) as p12v, \
                         tc.tile_pool(name="p12s", bufs=2) as p12s:
                        xc = p12.tile([P, KC, N], MMDT, tag="xc")
                        with tc.tile_pool(name="p1x", bufs=1) as p1x:
                            x_oth = p1x.tile([P, KC, TO], F32, tag="xoth")
                            dma(x_oth[:], xt_r)

                            for half, xsrc in ((0, x_own), (1, x_oth)):
                                psn = ps_norm.tile([P, TO], F32, tag="n")
                                for kc in range(KC):
                                    sq = p12s.tile([P, TO], BF16, tag="sq")
                                    nc.vector.tensor_mul(sq[:], xsrc[:, kc, :],
                                                         xsrc[:, kc, :])
                                    nc.tensor.matmul(psn[:], ones_c[:], sq[:],
                                                     start=(kc == 0),
                                                     stop=(kc == KC - 1))
                                sc = pixel_scale(p12s, psn, HID, eps4, "pnsc")
                                for kc in range(KC):
                                    cols = slice(half * TO, half * TO + TO)
                                    nc.vector.tensor_mul(xc[:, kc, cols],
                                                         xsrc[:, kc, :], sc[:])
                                    nc.vector.tensor_scalar(
                                        xc[:, kc, cols], xc[:, kc, cols],
                                        gs_sb[:, kc, 0:1], gs_sb[:, kc, 1:2],
                                        ALU.mult, ALU.add)

                        dbg_dump("xc", xc, KC, N)

                        q_sb = p_qkv.tile([P, KC, TO], BF16, tag="q")
                        k_sb = p_qkv.tile([P, KC, N], BF16, tag="k")
                        v_sb = p_qkv.tile([P, KC, NH, DH + 1], MMDT, tag="v")
                        nc.vector.tensor_copy(
                            v_sb[:, :, :, DH:DH + 1],
                            onecol[:].to_broadcast((P, KC, NH, 1)))

                        # ---- Q (+ per-head L2), K ----
                        for proj, out_t, nhalves in (("q", q_sb, 1),
                                                     ("k", k_sb, 2)):
                            off = 0 if proj == "q" else HID
                            for oc in range(KC):
                                wt = p12w.tile([P, KC, P], MMDT, tag="wqkv")
                                dma(wt[:], wqkv_r[:, :,
                                                  off + oc * P: off + (oc + 1) * P])
                                for half in range(nhalves):
                                    cols = slice(half * TO, half * TO + TO)
                                    psA = ps_acc.tile([P, TO], F32, tag="acc")
                                    for kc in range(KC):
                                        nc.tensor.matmul(psA[:], wt[:, kc, :],
                                                         xc[:, kc, cols],
                                                         start=(kc == 0),
                                                         stop=(kc == KC - 1))
                                    sqq = p12s.tile([P, TO], BF16, tag="sqq")
                                    nc.scalar.activation(sqq[:], psA[:],
                                                         AF.Square)
                                    psB = ps_norm.tile([P, TO], F32, tag="n")
                                    nc.tensor.matmul(psB[:], bd_c[:], sqq[:],
                                                     start=True, stop=True)
                                    hsc = p12s.tile([P, TO], F32, tag="hsc")
                                    nc.scalar.activation(hsc[:], psB[:], AF.Sqrt,
                                                         scale=1.0, bias=epsq[:])
                                    nc.vector.reciprocal(hsc[:], hsc[:])
                                    nc.vector.tensor_mul(out_t[:, oc, cols],
                                                         psA[:], hsc[:])

                        # ---- V (token-major, pixel-normed, ones col) ----
                        vstats = p12s.tile([P, KC, 2, nc.vector.BN_STATS_DIM],
                                           F32, tag="vstats")
                        for vc in range(2):
                            wt = p12v.tile([P, KC, TO], MMDT, tag="wv")
                            dma(wt[:], wqkv_r[:, :, 2 * HID + vc * TO:
                                              2 * HID + (vc + 1) * TO])
                            for tt in range(KC):
                                psV = ps_acc.tile([P, TO], F32, tag="acc")
                                for kc in range(KC):
                                    nc.tensor.matmul(
                                        psV[:], xc[:, kc, tt * P:(tt + 1) * P],
                                        wt[:, kc, :],
                                        start=(kc == 0), stop=(kc == KC - 1))
                                nc.vector.bn_stats(vstats[:, tt, vc, :], psV[:])
                                nc.scalar.copy(
                                    v_sb[:, tt, vc * 8:(vc + 1) * 8, 0:DH],
                                    psV[:].rearrange("p (h d) -> p h d", d=DH))
                        for tt in range(KC):
                            mv = p12s.tile([P, nc.vector.BN_AGGR_DIM], F32,
                                           tag="mv")
                            nc.vector.bn_aggr(mv[:], vstats[:, tt, :, :])
                            m2 = p12s.tile([P, 1], F32, tag="m2")
                            nc.vector.tensor_mul(m2[:], mv[:, 0:1], mv[:, 0:1])
                            nc.vector.tensor_add(m2[:], m2[:], mv[:, 1:2])
                            nc.scalar.activation(m2[:], m2[:], AF.Sqrt,
                                                 scale=1.0, bias=eps4[:])
                            nc.vector.reciprocal(m2[:], m2[:])
                            nc.vector.tensor_scalar_mul(
                                v_sb[:, tt, :, 0:DH], v_sb[:, tt, :, 0:DH],
                                m2[:])

                        dbg_dump("q", q_sb, KC, TO)
                        dbg_dump("k", k_sb, KC, N)
                        if _DBG:
                            vdst = dbg["vtm"].rearrange(
                                "(tt ki) d -> ki tt d", ki=P)
                            for tt in range(KC):
                                t = pdbg.tile([P, NH, DH], F32, tag="dbgv")
                                nc.vector.tensor_copy(t[:],
                                                      v_sb[:, tt, :, 0:DH])
                                dma(vdst[:, tt, :],
                                    t[:].rearrange("p h d -> p (h d)"))

                    # ============ phase 3: attention ============
                    ao = p_ao.tile([P, KC, TO], MMDT, tag="ao")
                    with tc.tile_pool(name="p3", bufs=1) as p3, \
                         tc.tile_pool(name="p3r", bufs=2) as p3r:
                        r_heads = p3.tile([NH, TO], F32, tag="rheads")
                        for h in range(NH):
                            hc, po = h // 2, DH * (h % 2)
                            pT = p3r.tile([P, KC, TO], MMDT, tag="pT")
                            for kt in range(KC):
                                ps_s = ps_acc.tile([P, TO], F32, tag="acc")
                                nc.tensor.matmul(
                                    ps_s[:],
                                    k_sb[po:po + DH, hc, kt * P:(kt + 1) * P],
                                    q_sb[po:po + DH, hc, :],
                                    start=True, stop=True)
                                nc.scalar.activation(pT[:, kt, :], ps_s[:],
                                                     AF.Exp, scale=0.125)
                            ps_av = ps_long.tile([DH + 1, TO], F32, tag="av")
                            for kt in range(KC):
                                nc.tensor.matmul(ps_av[:], v_sb[:, kt, h, :],
                                                 pT[:, kt, :],
                                                 start=(kt == 0),
                                                 stop=(kt == KC - 1))
                            rext = p3r.tile([DH + 1, TO], F32, tag="rext")
                            nc.scalar.copy(rext[DH:DH + 1, :],
                                           ps_av[DH:DH + 1, :])
                            dma(r_heads[h:h + 1, :], rext[DH:DH + 1, :])
                            stage = p3r.tile([DH, TO], MMDT, tag="stage")
                            nc.vector.tensor_copy(stage[:], ps_av[0:DH, :])
                            dma(ao[po:po + DH, hc, :], stage[:])

                        # 1/r, replicate via DRAM bounce, assemble, divide
                        nc.vector.reciprocal(r_heads[:], r_heads[:])
                        r_dram = drp.tile([NH, TO], F32, tag="rd")
                        dma(r_dram[:], r_heads[:])
                        rrep = p3.tile([P, KC, TO], F32, tag="rrep")
                        for kc in range(KC):
                            src = bass.AP(tensor=r_dram.tensor,
                                          offset=r_dram.offset + 2 * kc * TO,
                                          ap=[[TO, 2], [0, DH], [1, TO]])
                            dma(rrep[:, kc, :], src)
                        for kc in range(KC):
                            nc.vector.tensor_mul(ao[:, kc, :], ao[:, kc, :],
                                                 rrep[:, kc, :])
                        dbg_dump("ao", ao, KC, TO)

                # ============ phase 4: out-proj + residual ============
                with tc.tile_pool(name="p4", bufs=1) as p4, \
                     tc.tile_pool(name="p4w", bufs=3) as p4w, \
                     tc.tile_pool(name="p4s", bufs=2) as p4s:
                    yo = p4.tile([P, KC, TO], F32, tag="yo")
                    xc1 = pp.tile([P, KC, TO], F32, tag="x1")
                    psn = ps_long.tile([P, TO], F32, tag="av")
                    for oc in range(KC):
                        wt = p4w.tile([P, KC, P], MMDT, tag="wo")
                        dma(wt[:], wout_r[:, :, oc * P:(oc + 1) * P])
                        psA = ps_acc.tile([P, TO], F32, tag="acc")
                        for kc in range(KC):
                            nc.tensor.matmul(psA[:], wt[:, kc, :], ao[:, kc, :],
                                             start=(kc == 0), stop=(kc == KC - 1))
                        nc.scalar.copy(yo[:, oc, :], psA[:])
                        sqo = p4s.tile([P, TO], BF16, tag="sqo")
                        nc.vector.tensor_mul(sqo[:], yo[:, oc, :], yo[:, oc, :])
                        nc.tensor.matmul(psn[:], ones_c[:], sqo[:],
                                         start=(oc == 0), stop=(oc == KC - 1))
                    osc = pixel_scale(p4s, psn, HID, eps4, "osc")
                    nc.vector.tensor_scalar_mul(osc[:], osc[:], ag_c)
                    for oc in range(KC):
                        nc.vector.tensor_scalar_mul(xc1[:, oc, :],
                                                    x_own[:, oc, :], c1)
                        tmp = p4s.tile([P, TO], F32, tag="t4")
                        nc.vector.tensor_mul(tmp[:], yo[:, oc, :], osc[:])
                        nc.vector.tensor_add(xc1[:, oc, :], xc1[:, oc, :],
                                             tmp[:])
                    dbg_dump("x1", xc1, KC, TO)

            # ============ phase 5: x_cond2 ============
            with tc.tile_pool(name="p5", bufs=2) as p5:
                xc2 = pp.tile([P, KC, TO], MMDT, tag="xc2")
                psn = ps_norm.tile([P, TO], F32, tag="n")
                for kc in range(KC):
                    sq = p5.tile([P, TO], BF16, tag="sq5")
                    nc.vector.tensor_mul(sq[:], xc1[:, kc, :], xc1[:, kc, :])
                    nc.tensor.matmul(psn[:], ones_c[:], sq[:],
                                     start=(kc == 0), stop=(kc == KC - 1))
                sc = pixel_scale(p5, psn, HID, eps4, "sc5")
                for kc in range(KC):
                    nc.vector.tensor_mul(xc2[:, kc, :], xc1[:, kc, :], sc[:])
                    nc.vector.tensor_scalar(xc2[:, kc, :], xc2[:, kc, :],
                                            gs_sb[:, kc, 0:1], gs_sb[:, kc, 1:2],
                                            ALU.mult, ALU.add)

            # ============ phase 6+7: MLP ============
            with tc.tile_pool(name="p67", bufs=1) as p67:
                h_sb = p67.tile([P, MC, TO], MMDT, tag="h")
                with tc.tile_pool(name="p6w", bufs=3) as p6w, \
                     tc.tile_pool(name="p6s", bufs=2) as p6s:
                    psn = ps_long.tile([P, TO], F32, tag="av")
                    for oc in range(MC):
                        wt = p6w.tile([P, KC, P], MMDT, tag="w1")
                        dma(wt[:], w1_r[:, :, oc * P:(oc + 1) * P])
                        psA = ps_acc.tile([P, TO], F32, tag="acc")
                        for kc in range(KC):
                            nc.tensor.matmul(psA[:], wt[:, kc, :], xc2[:, kc, :],
                                             start=(kc == 0), stop=(kc == KC - 1))
                        nc.scalar.copy(h_sb[:, oc, :], psA[:])
                        sqh = p6s.tile([P, TO], BF16, tag="sqh")
                        nc.vector.tensor_mul(sqh[:], h_sb[:, oc, :],
                                             h_sb[:, oc, :])
                        nc.tensor.matmul(psn[:], ones_c[:], sqh[:],
                                         start=(oc == 0), stop=(oc == MC - 1))
                    msc = pixel_scale(p6s, psn, MLP, eps4, "msc")
                    for oc in range(MC):
                        nc.vector.tensor_mul(h_sb[:, oc, :], h_sb[:, oc, :],
                                             msc[:])
                        nc.scalar.activation(h_sb[:, oc, :], h_sb[:, oc, :],
                                             AF.Silu)
                    dbg_dump("hmid", h_sb, MC, TO)

                with tc.tile_pool(name="p7", bufs=1) as p7, \
                     tc.tile_pool(name="p7w", bufs=2) as p7w, \
                     tc.tile_pool(name="p7s", bufs=2) as p7s:
                    ym = p7.tile([P, KC, TO], F32, tag="ym")
                    psn = ps_long.tile([P, TO], F32, tag="av")
                    for oc in range(KC):
                        wt = p7w.tile([P, MC, P], MMDT, tag="w2")
                        dma(wt[:], w2_r[:, :, oc * P:(oc + 1) * P])
                        psA = ps_acc.tile([P, TO], F32, tag="acc")
                        for kc in range(MC):
                            nc.tensor.matmul(psA[:], wt[:, kc, :], h_sb[:, kc, :],
                                             start=(kc == 0), stop=(kc == MC - 1))
                        nc.scalar.copy(ym[:, oc, :], psA[:])
                        sqm = p7s.tile([P, TO], BF16, tag="sqm")
                        nc.vector.tensor_mul(sqm[:], ym[:, oc, :], ym[:, oc, :])
                        nc.tensor.matmul(psn[:], ones_c[:], sqm[:],
                                         start=(oc == 0), stop=(oc == KC - 1))
                    msc2 = pixel_scale(p7s, psn, HID, epsm, "msc2")
                    nc.vector.tensor_scalar_mul(msc2[:], msc2[:], mg_c)
                    out_t = p7.tile([P, KC, TO], F32, tag="out")
                    for oc in range(KC):
                        nc.vector.tensor_scalar_mul(out_t[:, oc, :],
                                                    xc1[:, oc, :], c1)
                        tmp = p7s.tile([P, TO], F32, tag="t7")
                        nc.vector.tensor_mul(tmp[:], ym[:, oc, :], msc2[:])
                        nc.vector.tensor_add(out_t[:, oc, :], out_t[:, oc, :],
                                             tmp[:])
                        dma(y_r[:, oc, :], out_t[:, oc, :])
    nc.compile()
    return nc


def kernel(x, c, w_cond, w_qkv, w_out, w_mlp1, w_mlp2, attn_gain, mlp_gain):
    import ml_dtypes
    from concourse.bass_utils import run_bass_kernel_spmd

    (wn_qkv, wn_out, wn_mlp1, wn_mlp2, g1, shift, ag_c, mg_c, c1) = _host_prep(
        x, c, w_cond, w_qkv, w_out, w_mlp1, w_mlp2, attn_gain, mlp_gain)

    wdt = np.float32 if _MMDT_NAME == "f32r" else ml_dtypes.bfloat16
    wn_qkv = np.ascontiguousarray(wn_qkv.astype(wdt))
    wn_out = np.ascontiguousarray(wn_out.astype(wdt))
    wn_mlp1 = np.ascontiguousarray(wn_mlp1.astype(wdt))
    wn_mlp2 = np.ascontiguousarray(wn_mlp2.astype(wdt))
    ones_c = np.ones((P, P), dtype=ml_dtypes.bfloat16)
    bd_c = np.zeros((P, P), dtype=np.float32)
    bd_c[0:DH, 0:DH] = 1.0
    bd_c[DH:P, DH:P] = 1.0
    bd_c = bd_c.astype(ml_dtypes.bfloat16)

    x32 = np.asarray(x, np.float32)
    g1_32 = g1.astype(np.float32)
    sh_32 = shift.astype(np.float32)

    in_maps = []
    for core in range(NCORES):
        b, half = core // 2, core % 2
        xb = x32[b]                                    # [N, HID]
        own = np.ascontiguousarray(xb[half * TO:(half + 1) * TO].T)
        oth = np.ascontiguousarray(xb[(1 - half) * TO:(2 - half) * TO].T)
        gs = np.empty((P, KC, 2), np.float32)
        gs[:, :, 0] = g1_32[b].reshape(KC, P).T
        gs[:, :, 1] = sh_32[b].reshape(KC, P).T
        in_maps.append({
            "x_own": own, "x_oth": oth, "gs": gs,
            "w_qkv": wn_qkv, "w_out": wn_out,
            "w_mlp1": wn_mlp1, "w_mlp2": wn_mlp2,
            "ones_c": ones_c, "bd_c": bd_c,
        })

    nc = _build_nc(ag_c, mg_c, c1)
    res = run_bass_kernel_spmd(nc, in_maps, core_ids=list(range(NCORES)),
                               trace=_TRACE)
    if _TRACE and res.exec_time_ns is not None:
        print(f"HW exec time: {res.exec_time_ns} ns")

    out = np.empty((B, N, HID), np.float32)
    for core in range(NCORES):
        b, half = core // 2, core % 2
        out[b, half * TO:(half + 1) * TO, :] = res.results[core]["y"].T
    kernel._last_results = res
    return out


# revision 8
# speedup vs baseline: 1.0377x; 1.0377x over previous
"""Trainium2 Bass kernel for the DiT block (B=4, N=1024, HID=1024, NH=16, MLP=4096).

Sharding: 8 cores = 4 batches x 2 sequence halves. Each core computes the
full block for its 512 tokens (feature-major layout [hidden, tokens]); K/V
for the other half of its batch are recomputed locally (~14% extra FLOPs,
zero collectives). Host gathers/concats the per-core [1024, 512] outputs.

Matmuls run in float32r (TF32-like, 1 cycle/row for moving dim >= 256).
Weight normalization and the tiny conditioning matmul (0.014% of FLOPs)
are folded on the host in float64.
"""

import math
import os

import numpy as np

B, N, HID, NH = 4, 1024, 1024, 16
DH = HID // NH
MLP = 4 * HID
P = 128
KC = HID // P          # 8 hidden chunks
MC = MLP // P          # 32 mlp chunks
TO = N // 2            # 512 own tokens per core
NCORES = 8

_TRACE = os.environ.get("DIT_TRACE", "0") == "1"
_DBG = os.environ.get("DIT_DEBUG_OUTPUTS", "0") == "1"
_MMDT_NAME = os.environ.get("DIT_MMDT", "f32r")  # f32r | bf16


def _normalize_w(w):
    w = np.asarray(w, np.float64)
    norm = np.linalg.norm(w, axis=1, keepdims=True)
    alpha = math.sqrt(w.shape[1])
    wn = w / (norm * alpha + 1e-4)
    wn = wn / math.sqrt(w.shape[0])
    return wn


def _host_prep(x, c, w_cond, w_qkv, w_out, w_mlp1, w_mlp2, attn_gain, mlp_gain):
    """Float64 host-side folding: weight norms, conditioning, scalars."""
    wn_qkv = _normalize_w(w_qkv)
    wn_out = _normalize_w(w_out)
    wn_mlp1 = _normalize_w(w_mlp1)
    wn_mlp2 = _normalize_w(w_mlp2)
    wn_cond = _normalize_w(w_cond)

    c64 = np.asarray(c, np.float64)
    silu = c64 / (1.0 + np.exp(-c64))
    cc = (silu / 0.596) @ wn_cond              # [B, 2H]
    gain, shift = cc[:, :HID], cc[:, HID:]
    g1 = 1.0 + gain

    ag_c = float(np.exp(np.float64(attn_gain)) * 0.3 / math.sqrt(0.58))
    mg_c = float(np.exp(np.float64(mlp_gain)) * 0.3 / math.sqrt(0.58))
    c1 = float(0.7 / math.sqrt(0.58))
    return (wn_qkv, wn_out, wn_mlp1, wn_mlp2, g1, shift, ag_c, mg_c, c1)


def _build_nc(ag_c, mg_c, c1):
    import concourse.bass as bass
    import concourse.tile as tile
    from concourse import bacc, mybir

    MMDT = mybir.dt.float32r if _MMDT_NAME == "f32r" else mybir.dt.bfloat16
    BF16 = mybir.dt.bfloat16
    F32 = mybir.dt.float32
    AF = mybir.ActivationFunctionType
    ALU = mybir.AluOpType

    nc = bacc.Bacc()

    # ---- dram I/O ----
    x_own_d = nc.dram_tensor("x_own", [HID, TO], F32, kind="ExternalInput")
    x_oth_d = nc.dram_tensor("x_oth", [HID, TO], F32, kind="ExternalInput")
    gs_d = nc.dram_tensor("gs", [P, KC, 2], F32, kind="ExternalInput")
    wqkv_d = nc.dram_tensor("w_qkv", [HID, 3 * HID], MMDT, kind="ExternalInput")
    wout_d = nc.dram_tensor("w_out", [HID, HID], MMDT, kind="ExternalInput")
    w1_d = nc.dram_tensor("w_mlp1", [HID, MLP], MMDT, kind="ExternalInput")
    w2_d = nc.dram_tensor("w_mlp2", [MLP, HID], MMDT, kind="ExternalInput")
    ones_d = nc.dram_tensor("ones_c", [P, P], BF16, kind="ExternalInput")
    bd_d = nc.dram_tensor("bd_c", [P, P], BF16, kind="ExternalInput")
    y_d = nc.dram_tensor("y", [HID, TO], F32, kind="ExternalOutput")

    dbg = {}
    if _DBG:
        for nm, shp in [("xc", [HID, N]), ("q", [HID, TO]), ("k", [HID, N]),
                        ("ao", [HID, TO]), ("x1", [HID, TO]), ("hmid", [MLP, TO]),
                        ("vtm", [N, HID])]:
            dbg[nm] = nc.dram_tensor("dbg_" + nm, shp, F32, kind="ExternalOutput")

    xo_r = x_own_d.rearrange("(kc ki) t -> ki kc t", ki=P)
    xt_r = x_oth_d.rearrange("(kc ki) t -> ki kc t", ki=P)
    wqkv_r = wqkv_d.rearrange("(kc ki) m -> ki kc m", ki=P)
    wout_r = wout_d.rearrange("(kc ki) m -> ki kc m", ki=P)
    w1_r = w1_d.rearrange("(kc ki) m -> ki kc m", ki=P)
    w2_r = w2_d.rearrange("(kc ki) m -> ki kc m", ki=P)
    y_r = y_d.rearrange("(kc ki) t -> ki kc t", ki=P)

    dma = nc.gpsimd.dma_start

    with tile.TileContext(nc) as tc:
        with tc.tile_pool(name="pp", bufs=1) as pp, \
             tc.tile_pool(name="pdbg", bufs=2) as pdbg, \
             tc.tile_pool(name="ps_acc", bufs=4, space="PSUM") as ps_acc, \
             tc.tile_pool(name="ps_norm", bufs=2, space="PSUM") as ps_norm, \
             tc.tile_pool(name="ps_long", bufs=2, space="PSUM") as ps_long, \
             tc.tile_pool(name="dram", bufs=1, space="DRAM") as drp:

            # ---- persistent small constants ----
            ones_c = pp.tile([P, P], BF16, tag="ones")
            dma(ones_c[:], ones_d[:, :])
            bd_c = pp.tile([P, P], BF16, tag="bd")
            dma(bd_c[:], bd_d[:, :])
            gs_sb = pp.tile([P, KC, 2], F32, tag="gs")
            dma(gs_sb[:], gs_d[:, :, :])
            eps4 = pp.tile([P, 1], F32, tag="eps4")
            nc.vector.memset(eps4[:], 1e-4)
            epsq = pp.tile([P, 1], F32, tag="epsq")
            nc.vector.memset(epsq[:], 1e-10)
            epsm = pp.tile([P, 1], F32, tag="epsm")
            nc.vector.memset(epsm[:], 1e-4 * 0.596 * 0.596)
            onecol = pp.tile([P, 1], F32, tag="onecol")
            nc.vector.memset(onecol[:], 1.0)
            ln_ag = pp.tile([P, 1], F32, tag="lnag")
            nc.vector.memset(ln_ag[:], math.log(ag_c))
            ln_mg = pp.tile([P, 1], F32, tag="lnmg")
            nc.vector.memset(ln_mg[:], math.log(mg_c))

            x_own = pp.tile([P, KC, TO], F32, tag="xown")
            dma(x_own[:], xo_r)

            def pixel_scale(pool, psum, mean_div, eps_tile, tag, lnbias=None):
                """scale = g/sqrt(psum/mean_div + eps) via exp(-0.5*ln(x)+ln(g)),
                ACT-engine only (no slow DVE reciprocal)."""
                s = pool.tile([P, TO], F32, tag=tag)
                nc.scalar.activation(s[:], psum[:], AF.Ln,
                                     scale=1.0 / mean_div, bias=eps_tile[:])
                if lnbias is None:
                    nc.scalar.activation(s[:], s[:], AF.Exp, scale=-0.5)
                else:
                    nc.scalar.activation(s[:], s[:], AF.Exp, scale=-0.5,
                                         bias=lnbias[:])
                return s

            def dbg_dump(name, src, nchunks, cols):
                """Stream src [P, nchunks, cols] (any dtype) to dbg[name]."""
                if not _DBG:
                    return
                dst = dbg[name].rearrange("(kc ki) t -> ki kc t", ki=P)
                for ch in range(nchunks):
                    t = pdbg.tile([P, cols], F32, tag="dbgt")
                    nc.vector.tensor_copy(t[:], src[:, ch, :])
                    dma(dst[:, ch, :], t[:])

            with tc.tile_pool(name="p_ao", bufs=1) as p_ao:
                with tc.tile_pool(name="p_qkv", bufs=1) as p_qkv:

                    # ============ phase 1+2: x_cond, QKV ============
                    with tc.tile_pool(name="p12", bufs=1) as p12, \
                         tc.tile_pool(name="p12w", bufs=2) as p12w, \
                         tc.tile_pool(name="p12v", bufs=1) as p12v, \
                         tc.tile_pool(name="p12s", bufs=2) as p12s:
                        xc = p12.tile([P, KC, N], MMDT, tag="xc")
                        with tc.tile_pool(name="p1x", bufs=1) as p1x:
                            x_oth = p1x.tile([P, KC, TO], F32, tag="xoth")
                            dma(x_oth[:], xt_r)

                            for half, xsrc in ((0, x_own), (1, x_oth)):
                                psn = ps_norm.tile([P, TO], F32, tag="n")
                                for kc in range(KC):
                                    sq = p12s.tile([P, TO], BF16, tag="sq")
                                    nc.vector.tensor_mul(sq[:], xsrc[:, kc, :],
                                                         xsrc[:, kc, :])
                                    nc.tensor.matmul(psn[:], ones_c[:], sq[:],
                                                     start=(kc == 0),
                                                     stop=(kc == KC - 1))
                                sc = pixel_scale(p12s, psn, HID, eps4, "pnsc")
                                for kc in range(KC):
                                    cols = slice(half * TO, half * TO + TO)
                                    nc.vector.tensor_mul(xc[:, kc, cols],
                                                         xsrc[:, kc, :], sc[:])
                                    nc.vector.tensor_scalar(
                                        xc[:, kc, cols], xc[:, kc, cols],
                                        gs_sb[:, kc, 0:1], gs_sb[:, kc, 1:2],
                                        ALU.mult, ALU.add)

                        dbg_dump("xc", xc, KC, N)

                        q_sb = p_qkv.tile([P, KC, TO], BF16, tag="q")
                        k_sb = p_qkv.tile([P, KC, N], BF16, tag="k")
                        v_sb = p_qkv.tile([P, KC, NH, DH + 1], BF16, tag="v")
                        nc.vector.tensor_copy(
                            v_sb[:, :, :, DH:DH + 1],
                            onecol[:].to_broadcast((P, KC, NH, 1)))

                        # ---- Q (+ per-head L2), K ----
                        for proj, out_t, nhalves in (("q", q_sb, 1),
                                                     ("k", k_sb, 2)):
                            off = 0 if proj == "q" else HID
                            for oc in range(KC):
                                wt = p12w.tile([P, KC, P], MMDT, tag="wqkv")
                                dma(wt[:], wqkv_r[:, :,
                                                  off + oc * P: off + (oc + 1) * P])
                                for half in range(nhalves):
                                    cols = slice(half * TO, half * TO + TO)
                                    psA = ps_acc.tile([P, TO], F32, tag="acc")
                                    for kc in range(KC):
                                        nc.tensor.matmul(psA[:], wt[:, kc, :],
                                                         xc[:, kc, cols],
                                                         start=(kc == 0),
                                                         stop=(kc == KC - 1))
                                    sqq = p12s.tile([P, TO], BF16, tag="sqq")
                                    nc.scalar.activation(sqq[:], psA[:],
                                                         AF.Square)
                                    psB = ps_norm.tile([P, TO], F32, tag="n")
                                    nc.tensor.matmul(psB[:], bd_c[:], sqq[:],
                                                     start=True, stop=True)
                                    hsc = p12s.tile([P, TO], F32, tag="hsc")
                                    nc.scalar.activation(hsc[:], psB[:], AF.Ln,
                                                         scale=1.0, bias=epsq[:])
                                    nc.scalar.activation(hsc[:], hsc[:], AF.Exp,
                                                         scale=-0.5)
                                    nc.vector.tensor_mul(out_t[:, oc, cols],
                                                         psA[:], hsc[:])

                        # ---- V (token-major, pixel-normed, ones col) ----
                        vstats = p12s.tile([P, KC, 2, nc.vector.BN_STATS_DIM],
                                           F32, tag="vstats")
                        for vc in range(2):
                            wt = p12v.tile([P, KC, TO], MMDT, tag="wv")
                            dma(wt[:], wqkv_r[:, :, 2 * HID + vc * TO:
                                              2 * HID + (vc + 1) * TO])
                            for tt in range(KC):
                                psV = ps_acc.tile([P, TO], F32, tag="acc")
                                for kc in range(KC):
                                    nc.tensor.matmul(
                                        psV[:], xc[:, kc, tt * P:(tt + 1) * P],
                                        wt[:, kc, :],
                                        start=(kc == 0), stop=(kc == KC - 1))
                                nc.vector.bn_stats(vstats[:, tt, vc, :], psV[:])
                                nc.scalar.copy(
                                    v_sb[:, tt, vc * 8:(vc + 1) * 8, 0:DH],
                                    psV[:].rearrange("p (h d) -> p h d", d=DH))
                        for tt in range(KC):
                            mv = p12s.tile([P, nc.vector.BN_AGGR_DIM], F32,
                                           tag="mv")
                            nc.vector.bn_aggr(mv[:], vstats[:, tt, :, :])
                            m2 = p12s.tile([P, 1], F32, tag="m2")
                            nc.vector.tensor_mul(m2[:], mv[:, 0:1], mv[:, 0:1])
                            nc.vector.tensor_add(m2[:], m2[:], mv[:, 1:2])
                            nc.scalar.activation(m2[:], m2[:], AF.Sqrt,
                                                 scale=1.0, bias=eps4[:])
                            nc.vector.reciprocal(m2[:], m2[:])
                            nc.vector.tensor_scalar_mul(
                                v_sb[:, tt, :, 0:DH], v_sb[:, tt, :, 0:DH],
                                m2[:])

                        dbg_dump("q", q_sb, KC, TO)
                        dbg_dump("k", k_sb, KC, N)
                        if _DBG:
                            vdst = dbg["vtm"].rearrange(
                                "(tt ki) d -> ki tt d", ki=P)
                            for tt in range(KC):
                                t = pdbg.tile([P, NH, DH], F32, tag="dbgv")
                                nc.vector.tensor_copy(t[:],
                                                      v_sb[:, tt, :, 0:DH])
                                dma(vdst[:, tt, :],
                                    t[:].rearrange("p h d -> p (h d)"))

                    # ============ phase 3: attention ============
                    ao = p_ao.tile([P, KC, TO], MMDT, tag="ao")
                    with tc.tile_pool(name="p3", bufs=1) as p3, \
                         tc.tile_pool(name="p3r", bufs=2) as p3r:
                        r_heads = p3.tile([NH, TO], F32, tag="rheads")
                        for h in range(NH):
                            hc, po = h // 2, DH * (h % 2)
                            pT = p3r.tile([P, KC, TO], BF16, tag="pT")
                            for kt in range(KC):
                                ps_s = ps_acc.tile([P, TO], F32, tag="acc")
                                nc.tensor.matmul(
                                    ps_s[:],
                                    k_sb[po:po + DH, hc, kt * P:(kt + 1) * P],
                                    q_sb[po:po + DH, hc, :],
                                    start=True, stop=True)
                                nc.scalar.activation(pT[:, kt, :], ps_s[:],
                                                     AF.Exp, scale=0.125)
                            ps_av = ps_long.tile([DH + 1, TO], F32, tag="av")
                            for kt in range(KC):
                                nc.tensor.matmul(ps_av[:], v_sb[:, kt, h, :],
                                                 pT[:, kt, :],
                                                 start=(kt == 0),
                                                 stop=(kt == KC - 1))
                            rext = p3r.tile([DH + 1, TO], F32, tag="rext")
                            nc.scalar.copy(rext[DH:DH + 1, :],
                                           ps_av[DH:DH + 1, :])
                            dma(r_heads[h:h + 1, :], rext[DH:DH + 1, :])
                            stage = p3r.tile([DH, TO], MMDT, tag="stage")
                            nc.vector.tensor_copy(stage[:], ps_av[0:DH, :])
                            dma(ao[po:po + DH, hc, :], stage[:])

                        # 1/r, replicate via DRAM bounce, assemble, divide
                        nc.scalar.activation(r_heads[:], r_heads[:], AF.Ln)
                        nc.scalar.activation(r_heads[:], r_heads[:], AF.Exp,
                                             scale=-1.0)
                        r_dram = drp.tile([NH, TO], F32, tag="rd")
                        dma(r_dram[:], r_heads[:])
                        rrep = p3.tile([P, KC, TO], F32, tag="rrep")
                        for kc in range(KC):
                            src = bass.AP(tensor=r_dram.tensor,
                                          offset=r_dram.offset + 2 * kc * TO,
                                          ap=[[TO, 2], [0, DH], [1, TO]])
                            dma(rrep[:, kc, :], src)
                        for kc in range(KC):
                            nc.vector.tensor_mul(ao[:, kc, :], ao[:, kc, :],
                                                 rrep[:, kc, :])
                        dbg_dump("ao", ao, KC, TO)

                # ============ phase 4: out-proj + residual ============
                with tc.tile_pool(name="p4", bufs=1) as p4, \
                     tc.tile_pool(name="p4w", bufs=3) as p4w, \
                     tc.tile_pool(name="p4s", bufs=2) as p4s:
                    yo = p4.tile([P, KC, TO], F32, tag="yo")
                    xc1 = pp.tile([P, KC, TO], F32, tag="x1")
                    psn = ps_long.tile([P, TO], F32, tag="av")
                    for oc in range(KC):
                        wt = p4w.tile([P, KC, P], MMDT, tag="wo")
                        dma(wt[:], wout_r[:, :, oc * P:(oc + 1) * P])
                        psA = ps_acc.tile([P, TO], F32, tag="acc")
                        for kc in range(KC):
                            nc.tensor.matmul(psA[:], wt[:, kc, :], ao[:, kc, :],
                                             start=(kc == 0), stop=(kc == KC - 1))
                        nc.scalar.copy(yo[:, oc, :], psA[:])
                        sqo = p4s.tile([P, TO], BF16, tag="sqo")
                        nc.vector.tensor_mul(sqo[:], yo[:, oc, :], yo[:, oc, :])
                        nc.tensor.matmul(psn[:], ones_c[:], sqo[:],
                                         start=(oc == 0), stop=(oc == KC - 1))
                    osc = pixel_scale(p4s, psn, HID, eps4, "osc", lnbias=ln_ag)
                    for oc in range(KC):
                        nc.vector.tensor_scalar_mul(xc1[:, oc, :],
                                                    x_own[:, oc, :], c1)
                        tmp = p4s.tile([P, TO], F32, tag="t4")
                        nc.vector.tensor_mul(tmp[:], yo[:, oc, :], osc[:])
                        nc.vector.tensor_add(xc1[:, oc, :], xc1[:, oc, :],
                                             tmp[:])
                    dbg_dump("x1", xc1, KC, TO)

            # ============ phase 5: x_cond2 ============
            with tc.tile_pool(name="p5", bufs=2) as p5:
                xc2 = pp.tile([P, KC, TO], MMDT, tag="xc2")
                psn = ps_norm.tile([P, TO], F32, tag="n")
                for kc in range(KC):
                    sq = p5.tile([P, TO], BF16, tag="sq5")
                    nc.vector.tensor_mul(sq[:], xc1[:, kc, :], xc1[:, kc, :])
                    nc.tensor.matmul(psn[:], ones_c[:], sq[:],
                                     start=(kc == 0), stop=(kc == KC - 1))
                sc = pixel_scale(p5, psn, HID, eps4, "sc5")
                for kc in range(KC):
                    nc.vector.tensor_mul(xc2[:, kc, :], xc1[:, kc, :], sc[:])
                    nc.vector.tensor_scalar(xc2[:, kc, :], xc2[:, kc, :],
                                            gs_sb[:, kc, 0:1], gs_sb[:, kc, 1:2],
                                            ALU.mult, ALU.add)

            # ============ phase 6+7: MLP ============
            with tc.tile_pool(name="p67", bufs=1) as p67:
                h_sb = p67.tile([P, MC, TO], MMDT, tag="h")
                with tc.tile_pool(name="p6w", bufs=3) as p6w, \
                     tc.tile_pool(name="p6s", bufs=2) as p6s:
                    psn = ps_long.tile([P, TO], F32, tag="av")
                    for oc in range(MC):
                        wt = p6w.tile([P, KC, P], MMDT, tag="w1")
                        dma(wt[:], w1_r[:, :, oc * P:(oc + 1) * P])
                        psA = ps_acc.tile([P, TO], F32, tag="acc")
                        for kc in range(KC):
                            nc.tensor.matmul(psA[:], wt[:, kc, :], xc2[:, kc, :],
                                             start=(kc == 0), stop=(kc == KC - 1))
                        nc.scalar.copy(h_sb[:, oc, :], psA[:])
                        sqh = p6s.tile([P, TO], BF16, tag="sqh")
                        nc.vector.tensor_mul(sqh[:], h_sb[:, oc, :],
                                             h_sb[:, oc, :])
                        nc.tensor.matmul(psn[:], ones_c[:], sqh[:],
                                         start=(oc == 0), stop=(oc == MC - 1))
                    msc = pixel_scale(p6s, psn, MLP, eps4, "msc")
                    for oc in range(MC):
                        nc.vector.tensor_mul(h_sb[:, oc, :], h_sb[:, oc, :],
                                             msc[:])
                        nc.scalar.activation(h_sb[:, oc, :], h_sb[:, oc, :],
                                             AF.Silu)
                    dbg_dump("hmid", h_sb, MC, TO)

                with tc.tile_pool(name="p7", bufs=1) as p7, \
                     tc.tile_pool(name="p7w", bufs=2) as p7w, \
                     tc.tile_pool(name="p7s", bufs=2) as p7s:
                    ym = p7.tile([P, KC, TO], F32, tag="ym")
                    psn = ps_long.tile([P, TO], F32, tag="av")
                    for oc in range(KC):
                        wt = p7w.tile([P, MC, P], MMDT, tag="w2")
                        dma(wt[:], w2_r[:, :, oc * P:(oc + 1) * P])
                        psA = ps_acc.tile([P, TO], F32, tag="acc")
                        for kc in range(MC):
                            nc.tensor.matmul(psA[:], wt[:, kc, :], h_sb[:, kc, :],
                                             start=(kc == 0), stop=(kc == MC - 1))
                        nc.scalar.copy(ym[:, oc, :], psA[:])
                        sqm = p7s.tile([P, TO], BF16, tag="sqm")
                        nc.vector.tensor_mul(sqm[:], ym[:, oc, :], ym[:, oc, :])
                        nc.tensor.matmul(psn[:], ones_c[:], sqm[:],
                                         start=(oc == 0), stop=(oc == KC - 1))
                    msc2 = pixel_scale(p7s, psn, HID, epsm, "msc2",
                                       lnbias=ln_mg)
                    out_t = p7.tile([P, KC, TO], F32, tag="out")
                    for oc in range(KC):
                        nc.vector.tensor_scalar_mul(out_t[:, oc, :],
                                                    xc1[:, oc, :], c1)
                        tmp = p7s.tile([P, TO], F32, tag="t7")
                        nc.vector.tensor_mul(tmp[:], ym[:, oc, :], msc2[:])
                        nc.vector.tensor_add(out_t[:, oc, :], out_t[:, oc, :],
                                             tmp[:])
                        dma(y_r[:, oc, :], out_t[:, oc, :])
    nc.compile()
    return nc


def kernel(x, c, w_cond, w_qkv, w_out, w_mlp1, w_mlp2, attn_gain, mlp_gain):
    import ml_dtypes
    from concourse.bass_utils import run_bass_kernel_spmd

    (wn_qkv, wn_out, wn_mlp1, wn_mlp2, g1, shift, ag_c, mg_c, c1) = _host_prep(
        x, c, w_cond, w_qkv, w_out, w_mlp1, w_mlp2, attn_gain, mlp_gain)

    wdt = np.float32 if _MMDT_NAME == "f32r" else ml_dtypes.bfloat16
    wn_qkv = np.ascontiguousarray(wn_qkv.astype(wdt))
    wn_out = np.ascontiguousarray(wn_out.astype(wdt))
    wn_mlp1 = np.ascontiguousarray(wn_mlp1.astype(wdt))
    wn_mlp2 = np.ascontiguousarray(wn_mlp2.astype(wdt))
    ones_c = np.ones((P, P), dtype=ml_dtypes.bfloat16)
    bd_c = np.zeros((P, P), dtype=np.float32)
    bd_c[0:DH, 0:DH] = 1.0
    bd_c[DH:P, DH:P] = 1.0
    bd_c = bd_c.astype(ml_dtypes.bfloat16)

    x32 = np.asarray(x, np.float32)
    g1_32 = g1.astype(np.float32)
    sh_32 = shift.astype(np.float32)

    in_maps = []
    for core in range(NCORES):
        b, half = core // 2, core % 2
        xb = x32[b]                                    # [N, HID]
        own = np.ascontiguousarray(xb[half * TO:(half + 1) * TO].T)
        oth = np.ascontiguousarray(xb[(1 - half) * TO:(2 - half) * TO].T)
        gs = np.empty((P, KC, 2), np.float32)
        gs[:, :, 0] = g1_32[b].reshape(KC, P).T
        gs[:, :, 1] = sh_32[b].reshape(KC, P).T
        in_maps.append({
            "x_own": own, "x_oth": oth, "gs": gs,
            "w_qkv": wn_qkv, "w_out": wn_out,
            "w_mlp1": wn_mlp1, "w_mlp2": wn_mlp2,
            "ones_c": ones_c, "bd_c": bd_c,
        })

    nc = _build_nc(ag_c, mg_c, c1)
    res = run_bass_kernel_spmd(nc, in_maps, core_ids=list(range(NCORES)),
                               trace=_TRACE)
    if _TRACE and res.exec_time_ns is not None:
        print(f"HW exec time: {res.exec_time_ns} ns")

    out = np.empty((B, N, HID), np.float32)
    for core in range(NCORES):
        b, half = core // 2, core % 2
        out[b, half * TO:(half + 1) * TO, :] = res.results[core]["y"].T
    kernel._last_results = res
    return out


# revision 11
# speedup vs baseline: 1.0724x; 1.0334x over previous
"""Trainium2 Bass kernel for the DiT block (B=4, N=1024, HID=1024, NH=16, MLP=4096).

Sharding: 8 cores = 4 batches x 2 sequence halves. Each core computes the
full block for its 512 tokens (feature-major layout [hidden, tokens]); K/V
for the other half of its batch are recomputed locally (~14% extra FLOPs,
zero collectives). Host gathers/concats the per-core [1024, 512] outputs.

Matmuls run in float32r (TF32-like, 1 cycle/row for moving dim >= 256).
Weight normalization and the tiny conditioning matmul (0.014% of FLOPs)
are folded on the host in float64.
"""

import math
import os

import numpy as np

B, N, HID, NH = 4, 1024, 1024, 16
DH = HID // NH
MLP = 4 * HID
P = 128
KC = HID // P          # 8 hidden chunks
MC = MLP // P          # 32 mlp chunks
TO = N // 2            # 512 own tokens per core
NCORES = 8

_TRACE = os.environ.get("DIT_TRACE", "0") == "1"
_DBG = os.environ.get("DIT_DEBUG_OUTPUTS", "0") == "1"
_MMDT_NAME = os.environ.get("DIT_MMDT", "f32r")  # f32r | bf16


def _normalize_w(w):
    w = np.asarray(w, np.float64)
    norm = np.linalg.norm(w, axis=1, keepdims=True)
    alpha = math.sqrt(w.shape[1])
    wn = w / (norm * alpha + 1e-4)
    wn = wn / math.sqrt(w.shape[0])
    return wn


def _host_prep(x, c, w_cond, w_qkv, w_out, w_mlp1, w_mlp2, attn_gain, mlp_gain):
    """Float64 host-side folding: weight norms, conditioning, scalars."""
    wn_qkv = _normalize_w(w_qkv)
    wn_out = _normalize_w(w_out)
    wn_mlp1 = _normalize_w(w_mlp1)
    wn_mlp2 = _normalize_w(w_mlp2)
    wn_cond = _normalize_w(w_cond)

    c64 = np.asarray(c, np.float64)
    silu = c64 / (1.0 + np.exp(-c64))
    cc = (silu / 0.596) @ wn_cond              # [B, 2H]
    gain, shift = cc[:, :HID], cc[:, HID:]
    g1 = 1.0 + gain

    ag_c = float(np.exp(np.float64(attn_gain)) * 0.3 / math.sqrt(0.58))
    mg_c = float(np.exp(np.float64(mlp_gain)) * 0.3 / math.sqrt(0.58))
    c1 = float(0.7 / math.sqrt(0.58))
    return (wn_qkv, wn_out, wn_mlp1, wn_mlp2, g1, shift, ag_c, mg_c, c1)


def _build_nc(ag_c, mg_c, c1):
    import concourse.bass as bass
    import concourse.tile as tile
    from concourse import bacc, mybir

    MMDT = mybir.dt.float32r if _MMDT_NAME == "f32r" else mybir.dt.bfloat16
    BF16 = mybir.dt.bfloat16
    F32 = mybir.dt.float32
    AF = mybir.ActivationFunctionType
    ALU = mybir.AluOpType

    import concourse.bacc as _bacc_mod
    from concourse.hw_specs import get_activation_tables as _gat

    def _steered_tables(arch):
        # Keep act_info.json set order (act_func_set_id is positional), but
        # strip our funcs from all sets except the two we want, so the
        # selector binds Copy/Exp/Ln/Square to one set and Silu to another
        # (minimizes ACT table reloads).
        t = dict(_gat(arch))
        keep_all = "natural_log_exp_and_others"
        steer = {f for f in t.get(keep_all, ())
                 if f.name in ("Copy", "Exp", "Ln", "Square", "MemsetZero",
                               "Identity")}
        out = {}
        for name, funcs in t.items():
            if name == keep_all:
                out[name] = funcs
            elif name == "silu_and_others":
                out[name] = {f for f in funcs if f.name not in
                             ("Copy", "Exp", "Ln", "Square")}
            else:
                out[name] = funcs - steer
        return out

    _bacc_mod.get_activation_tables = _steered_tables

    nc = bacc.Bacc()

    # ---- dram I/O ----
    x_own_d = nc.dram_tensor("x_own", [HID, TO], F32, kind="ExternalInput")
    x_oth_d = nc.dram_tensor("x_oth", [HID, TO], F32, kind="ExternalInput")
    gs_d = nc.dram_tensor("gs", [P, KC, 2], F32, kind="ExternalInput")
    wqkv_d = nc.dram_tensor("w_qkv", [HID, 3 * HID], MMDT, kind="ExternalInput")
    wout_d = nc.dram_tensor("w_out", [HID, HID], MMDT, kind="ExternalInput")
    w1_d = nc.dram_tensor("w_mlp1", [HID, MLP], MMDT, kind="ExternalInput")
    w2_d = nc.dram_tensor("w_mlp2", [MLP, HID], MMDT, kind="ExternalInput")
    ones_d = nc.dram_tensor("ones_c", [P, P], BF16, kind="ExternalInput")
    bd_d = nc.dram_tensor("bd_c", [P, P], BF16, kind="ExternalInput")
    y_d = nc.dram_tensor("y", [HID, TO], F32, kind="ExternalOutput")

    dbg = {}
    if _DBG:
        for nm, shp in [("xc", [HID, N]), ("q", [HID, TO]), ("k", [HID, N]),
                        ("ao", [HID, TO]), ("x1", [HID, TO]), ("hmid", [MLP, TO]),
                        ("vtm", [N, HID])]:
            dbg[nm] = nc.dram_tensor("dbg_" + nm, shp, F32, kind="ExternalOutput")

    xo_r = x_own_d.rearrange("(kc ki) t -> ki kc t", ki=P)
    xt_r = x_oth_d.rearrange("(kc ki) t -> ki kc t", ki=P)
    wqkv_r = wqkv_d.rearrange("(kc ki) m -> ki kc m", ki=P)
    wout_r = wout_d.rearrange("(kc ki) m -> ki kc m", ki=P)
    w1_r = w1_d.rearrange("(kc ki) m -> ki kc m", ki=P)
    w2_r = w2_d.rearrange("(kc ki) m -> ki kc m", ki=P)
    y_r = y_d.rearrange("(kc ki) t -> ki kc t", ki=P)

    dma = nc.gpsimd.dma_start

    with tile.TileContext(nc) as tc:
        with tc.tile_pool(name="pp", bufs=1) as pp, \
             tc.tile_pool(name="pdbg", bufs=2) as pdbg, \
             tc.tile_pool(name="ps_acc", bufs=4, space="PSUM") as ps_acc, \
             tc.tile_pool(name="ps_norm", bufs=2, space="PSUM") as ps_norm, \
             tc.tile_pool(name="ps_long", bufs=2, space="PSUM") as ps_long, \
             tc.tile_pool(name="dram", bufs=1, space="DRAM") as drp:

            # ---- persistent small constants ----
            ones_c = pp.tile([P, P], BF16, tag="ones")
            dma(ones_c[:], ones_d[:, :])
            bd_c = pp.tile([P, P], BF16, tag="bd")
            dma(bd_c[:], bd_d[:, :])
            gs_sb = pp.tile([P, KC, 2], F32, tag="gs")
            dma(gs_sb[:], gs_d[:, :, :])
            eps4 = pp.tile([P, 1], F32, tag="eps4")
            nc.vector.memset(eps4[:], 1e-4)
            epsq = pp.tile([P, 1], F32, tag="epsq")
            nc.vector.memset(epsq[:], 1e-10)
            epsm = pp.tile([P, 1], F32, tag="epsm")
            nc.vector.memset(epsm[:], 1e-4 * 0.596 * 0.596)
            onecol = pp.tile([P, 1], F32, tag="onecol")
            nc.vector.memset(onecol[:], 1.0)
            ln_ag = pp.tile([P, 1], F32, tag="lnag")
            nc.vector.memset(ln_ag[:], math.log(ag_c))
            ln_mg = pp.tile([P, 1], F32, tag="lnmg")
            nc.vector.memset(ln_mg[:], math.log(mg_c))

            x_own = pp.tile([P, KC, TO], F32, tag="xown")
            dma(x_own[:], xo_r)

            def pixel_scale(pool, psum, mean_div, eps_tile, tag, lnbias=None):
                """scale = g/sqrt(psum/mean_div + eps) via exp(-0.5*ln(x)+ln(g)),
                ACT-engine only (no slow DVE reciprocal)."""
                s = pool.tile([P, TO], F32, tag=tag)
                nc.scalar.activation(s[:], psum[:], AF.Ln,
                                     scale=1.0 / mean_div, bias=eps_tile[:])
                if lnbias is None:
                    nc.scalar.activation(s[:], s[:], AF.Exp, scale=-0.5)
                else:
                    nc.scalar.activation(s[:], s[:], AF.Exp, scale=-0.5,
                                         bias=lnbias[:])
                return s

            def dbg_dump(name, src, nchunks, cols):
                """Stream src [P, nchunks, cols] (any dtype) to dbg[name]."""
                if not _DBG:
                    return
                dst = dbg[name].rearrange("(kc ki) t -> ki kc t", ki=P)
                for ch in range(nchunks):
                    t = pdbg.tile([P, cols], F32, tag="dbgt")
                    nc.vector.tensor_copy(t[:], src[:, ch, :])
                    dma(dst[:, ch, :], t[:])

            with tc.tile_pool(name="p_ao", bufs=1) as p_ao:
                with tc.tile_pool(name="p_qkv", bufs=1) as p_qkv:

                    # ============ phase 1+2: x_cond, QKV ============
                    with tc.tile_pool(name="p12", bufs=1) as p12, \
                         tc.tile_pool(name="p12w", bufs=2) as p12w, \
                         tc.tile_pool(name="p12v", bufs=1) as p12v, \
                         tc.tile_pool(name="p12s", bufs=2) as p12s:
                        xc = p12.tile([P, KC, N], MMDT, tag="xc")
                        with tc.tile_pool(name="p1x", bufs=1) as p1x:
                            x_oth = p1x.tile([P, KC, TO], F32, tag="xoth")
                            dma(x_oth[:], xt_r)

                            for half, xsrc in ((0, x_own), (1, x_oth)):
                                psn = ps_norm.tile([P, TO], F32, tag="n")
                                for kc in range(KC):
                                    sq = p12s.tile([P, TO], BF16, tag="sq")
                                    nc.vector.tensor_mul(sq[:], xsrc[:, kc, :],
                                                         xsrc[:, kc, :])
                                    nc.tensor.matmul(psn[:], ones_c[:], sq[:],
                                                     start=(kc == 0),
                                                     stop=(kc == KC - 1))
                                sc = pixel_scale(p12s, psn, HID, eps4, "pnsc")
                                for kc in range(KC):
                                    cols = slice(half * TO, half * TO + TO)
                                    nc.vector.tensor_mul(xc[:, kc, cols],
                                                         xsrc[:, kc, :], sc[:])
                                    nc.vector.tensor_scalar(
                                        xc[:, kc, cols], xc[:, kc, cols],
                                        gs_sb[:, kc, 0:1], gs_sb[:, kc, 1:2],
                                        ALU.mult, ALU.add)

                        dbg_dump("xc", xc, KC, N)

                        q_sb = p_qkv.tile([P, KC, TO], BF16, tag="q")
                        k_sb = p_qkv.tile([P, KC, N], BF16, tag="k")
                        v_sb = p_qkv.tile([P, KC, NH, DH + 1], BF16, tag="v")
                        nc.vector.tensor_copy(
                            v_sb[:, :, :, DH:DH + 1],
                            onecol[:].to_broadcast((P, KC, NH, 1)))

                        # ---- Q (+ per-head L2), K ----
                        for proj, out_t, nhalves in (("q", q_sb, 1),
                                                     ("k", k_sb, 2)):
                            off = 0 if proj == "q" else HID
                            for oc in range(KC):
                                wt = p12w.tile([P, KC, P], MMDT, tag="wqkv")
                                dma(wt[:], wqkv_r[:, :,
                                                  off + oc * P: off + (oc + 1) * P])
                                for half in range(nhalves):
                                    cols = slice(half * TO, half * TO + TO)
                                    psA = ps_acc.tile([P, TO], F32, tag="acc")
                                    for kc in range(KC):
                                        nc.tensor.matmul(psA[:], wt[:, kc, :],
                                                         xc[:, kc, cols],
                                                         start=(kc == 0),
                                                         stop=(kc == KC - 1))
                                    sqq = p12s.tile([P, TO], BF16, tag="sqq")
                                    nc.scalar.activation(sqq[:], psA[:],
                                                         AF.Square)
                                    psB = ps_norm.tile([P, TO], F32, tag="n")
                                    nc.tensor.matmul(psB[:], bd_c[:], sqq[:],
                                                     start=True, stop=True)
                                    hsc = p12s.tile([P, TO], F32, tag="hsc")
                                    nc.scalar.activation(hsc[:], psB[:], AF.Ln,
                                                         scale=1.0, bias=epsq[:])
                                    nc.scalar.activation(hsc[:], hsc[:], AF.Exp,
                                                         scale=-0.5)
                                    nc.vector.tensor_mul(out_t[:, oc, cols],
                                                         psA[:], hsc[:])

                        # ---- V (token-major, pixel-normed, ones col) ----
                        vstats = p12s.tile([P, KC, 2, nc.vector.BN_STATS_DIM],
                                           F32, tag="vstats")
                        for vc in range(2):
                            wt = p12v.tile([P, KC, TO], MMDT, tag="wv")
                            dma(wt[:], wqkv_r[:, :, 2 * HID + vc * TO:
                                              2 * HID + (vc + 1) * TO])
                            for tt in range(KC):
                                psV = ps_acc.tile([P, TO], F32, tag="acc")
                                for kc in range(KC):
                                    nc.tensor.matmul(
                                        psV[:], xc[:, kc, tt * P:(tt + 1) * P],
                                        wt[:, kc, :],
                                        start=(kc == 0), stop=(kc == KC - 1))
                                nc.vector.bn_stats(vstats[:, tt, vc, :], psV[:])
                                nc.scalar.copy(
                                    v_sb[:, tt, vc * 8:(vc + 1) * 8, 0:DH],
                                    psV[:].rearrange("p (h d) -> p h d", d=DH))
                        for tt in range(KC):
                            mv = p12s.tile([P, nc.vector.BN_AGGR_DIM], F32,
                                           tag="mv")
                            nc.vector.bn_aggr(mv[:], vstats[:, tt, :, :])
                            m2 = p12s.tile([P, 1], F32, tag="m2")
                            nc.vector.tensor_mul(m2[:], mv[:, 0:1], mv[:, 0:1])
                            nc.vector.tensor_add(m2[:], m2[:], mv[:, 1:2])
                            nc.scalar.activation(m2[:], m2[:], AF.Ln,
                                                 scale=1.0, bias=eps4[:])
                            nc.scalar.activation(m2[:], m2[:], AF.Exp,
                                                 scale=-0.5)
                            nc.vector.tensor_scalar_mul(
                                v_sb[:, tt, :, 0:DH], v_sb[:, tt, :, 0:DH],
                                m2[:])

                        dbg_dump("q", q_sb, KC, TO)
                        dbg_dump("k", k_sb, KC, N)
                        if _DBG:
                            vdst = dbg["vtm"].rearrange(
                                "(tt ki) d -> ki tt d", ki=P)
                            for tt in range(KC):
                                t = pdbg.tile([P, NH, DH], F32, tag="dbgv")
                                nc.vector.tensor_copy(t[:],
                                                      v_sb[:, tt, :, 0:DH])
                                dma(vdst[:, tt, :],
                                    t[:].rearrange("p h d -> p (h d)"))

                    # ============ phase 3: attention ============
                    ao = p_ao.tile([P, KC, TO], MMDT, tag="ao")
                    with tc.tile_pool(name="p3", bufs=1) as p3, \
                         tc.tile_pool(name="p3r", bufs=2) as p3r, \
                         tc.tile_pool(name="p3q", bufs=4) as p3q:
                        r_heads = p3.tile([NH, TO], F32, tag="rheads")
                        for hp in range(NH // 2):
                            hc = hp
                            pTs = []
                            for sub in range(2):
                                po = DH * sub
                                pT = p3r.tile([P, KC, TO], BF16, tag="pT")
                                pTs.append(pT)
                            # interleave the two heads' score MMs: they sit in
                            # disjoint row groups (partitions 0-63 / 64-127) so
                            # the PE array runs them concurrently
                            for kt in range(KC):
                                for sub in range(2):
                                    po = DH * sub
                                    ps_s = ps_acc.tile([P, TO], F32, tag="acc")
                                    nc.tensor.matmul(
                                        ps_s[:],
                                        k_sb[po:po + DH, hc,
                                             kt * P:(kt + 1) * P],
                                        q_sb[po:po + DH, hc, :],
                                        start=True, stop=True)
                                    nc.scalar.activation(pTs[sub][:, kt, :],
                                                         ps_s[:], AF.Exp,
                                                         scale=0.125)
                            for sub in range(2):
                                h = 2 * hp + sub
                                po = DH * sub
                                pT = pTs[sub]
                                ps_av = ps_long.tile([DH + 1, TO], F32,
                                                     tag="av")
                                for kt in range(KC):
                                    nc.tensor.matmul(ps_av[:],
                                                     v_sb[:, kt, h, :],
                                                     pT[:, kt, :],
                                                     start=(kt == 0),
                                                     stop=(kt == KC - 1))
                                rext = p3q.tile([DH + 1, TO], F32, tag="rext")
                                nc.scalar.copy(rext[DH:DH + 1, :],
                                               ps_av[DH:DH + 1, :])
                                dma(r_heads[h:h + 1, :], rext[DH:DH + 1, :])
                                stage = p3q.tile([DH, TO], MMDT, tag="stage")
                                nc.vector.tensor_copy(stage[:], ps_av[0:DH, :])
                                dma(ao[po:po + DH, hc, :], stage[:])

                        # 1/r, replicate via DRAM bounce, divide in fm layout
                        nc.scalar.activation(r_heads[:], r_heads[:], AF.Ln)
                        nc.scalar.activation(r_heads[:], r_heads[:], AF.Exp,
                                             scale=-1.0)
                        r_dram = drp.tile([NH, TO], F32, tag="rd")
                        dma(r_dram[:], r_heads[:])
                        rrep = p3.tile([P, KC, TO], F32, tag="rrep")
                        for kc in range(KC):
                            src = bass.AP(tensor=r_dram.tensor,
                                          offset=r_dram.offset + 2 * kc * TO,
                                          ap=[[TO, 2], [0, DH], [1, TO]])
                            dma(rrep[:, kc, :], src)
                        for kc in range(KC):
                            nc.vector.tensor_mul(ao[:, kc, :], ao[:, kc, :],
                                                 rrep[:, kc, :])
                        dbg_dump("ao", ao, KC, TO)

                # ============ phase 4: out-proj + residual ============
                with tc.tile_pool(name="p4", bufs=1) as p4, \
                     tc.tile_pool(name="p4w", bufs=3) as p4w, \
                     tc.tile_pool(name="p4s", bufs=2) as p4s:
                    yo = p4.tile([P, KC, TO], F32, tag="yo")
                    xc1 = pp.tile([P, KC, TO], F32, tag="x1")
                    psn = ps_long.tile([P, TO], F32, tag="av")
                    for oc in range(KC):
                        wt = p4w.tile([P, KC, P], MMDT, tag="wo")
                        dma(wt[:], wout_r[:, :, oc * P:(oc + 1) * P])
                        psA = ps_acc.tile([P, TO], F32, tag="acc")
                        for kc in range(KC):
                            nc.tensor.matmul(psA[:], wt[:, kc, :], ao[:, kc, :],
                                             start=(kc == 0), stop=(kc == KC - 1))
                        nc.scalar.copy(yo[:, oc, :], psA[:])
                        sqo = p4s.tile([P, TO], BF16, tag="sqo")
                        nc.vector.tensor_mul(sqo[:], yo[:, oc, :], yo[:, oc, :])
                        nc.tensor.matmul(psn[:], ones_c[:], sqo[:],
                                         start=(oc == 0), stop=(oc == KC - 1))
                    osc = pixel_scale(p4s, psn, HID, eps4, "osc", lnbias=ln_ag)
                    for oc in range(KC):
                        nc.vector.tensor_scalar_mul(xc1[:, oc, :],
                                                    x_own[:, oc, :], c1)
                        tmp = p4s.tile([P, TO], F32, tag="t4")
                        nc.vector.tensor_mul(tmp[:], yo[:, oc, :], osc[:])
                        nc.vector.tensor_add(xc1[:, oc, :], xc1[:, oc, :],
                                             tmp[:])
                    dbg_dump("x1", xc1, KC, TO)

            # ============ phase 5: x_cond2 ============
            with tc.tile_pool(name="p5", bufs=2) as p5:
                xc2 = pp.tile([P, KC, TO], MMDT, tag="xc2")
                psn = ps_norm.tile([P, TO], F32, tag="n")
                for kc in range(KC):
                    sq = p5.tile([P, TO], BF16, tag="sq5")
                    nc.vector.tensor_mul(sq[:], xc1[:, kc, :], xc1[:, kc, :])
                    nc.tensor.matmul(psn[:], ones_c[:], sq[:],
                                     start=(kc == 0), stop=(kc == KC - 1))
                sc = pixel_scale(p5, psn, HID, eps4, "sc5")
                for kc in range(KC):
                    nc.vector.tensor_mul(xc2[:, kc, :], xc1[:, kc, :], sc[:])
                    nc.vector.tensor_scalar(xc2[:, kc, :], xc2[:, kc, :],
                                            gs_sb[:, kc, 0:1], gs_sb[:, kc, 1:2],
                                            ALU.mult, ALU.add)

            # ============ phase 6+7: MLP ============
            with tc.tile_pool(name="p67", bufs=1) as p67:
                h_sb = p67.tile([P, MC, TO], MMDT, tag="h")
                with tc.tile_pool(name="p6w", bufs=3) as p6w, \
                     tc.tile_pool(name="p6s", bufs=2) as p6s:
                    psn = ps_long.tile([P, TO], F32, tag="av")
                    for oc in range(MC):
                        wt = p6w.tile([P, KC, P], MMDT, tag="w1")
                        dma(wt[:], w1_r[:, :, oc * P:(oc + 1) * P])
                        psA = ps_acc.tile([P, TO], F32, tag="acc")
                        for kc in range(KC):
                            nc.tensor.matmul(psA[:], wt[:, kc, :], xc2[:, kc, :],
                                             start=(kc == 0), stop=(kc == KC - 1))
                        nc.scalar.copy(h_sb[:, oc, :], psA[:])
                        sqh = p6s.tile([P, TO], BF16, tag="sqh")
                        nc.vector.tensor_mul(sqh[:], h_sb[:, oc, :],
                                             h_sb[:, oc, :])
                        nc.tensor.matmul(psn[:], ones_c[:], sqh[:],
                                         start=(oc == 0), stop=(oc == MC - 1))
                    msc = pixel_scale(p6s, psn, MLP, eps4, "msc")
                    for oc in range(MC):
                        nc.vector.tensor_mul(h_sb[:, oc, :], h_sb[:, oc, :],
                                             msc[:])
                        nc.scalar.activation(h_sb[:, oc, :], h_sb[:, oc, :],
                                             AF.Silu)
                    dbg_dump("hmid", h_sb, MC, TO)

                with tc.tile_pool(name="p7", bufs=1) as p7, \
                     tc.tile_pool(name="p7w", bufs=2) as p7w, \
                     tc.tile_pool(name="p7s", bufs=2) as p7s:
                    ym = p7.tile([P, KC, TO], F32, tag="ym")
                    psn = ps_long.tile([P, TO], F32, tag="av")
                    for oc in range(KC):
                        wt = p7w.tile([P, MC, P], MMDT, tag="w2")
                        dma(wt[:], w2_r[:, :, oc * P:(oc + 1) * P])
                        psA = ps_acc.tile([P, TO], F32, tag="acc")
                        for kc in range(MC):
                            nc.tensor.matmul(psA[:], wt[:, kc, :], h_sb[:, kc, :],
                                             start=(kc == 0), stop=(kc == MC - 1))
                        nc.scalar.copy(ym[:, oc, :], psA[:])
                        sqm = p7s.tile([P, TO], BF16, tag="sqm")
                        nc.vector.tensor_mul(sqm[:], ym[:, oc, :], ym[:, oc, :])
                        nc.tensor.matmul(psn[:], ones_c[:], sqm[:],
                                         start=(oc == 0), stop=(oc == KC - 1))
                    msc2 = pixel_scale(p7s, psn, HID, epsm, "msc2",
                                       lnbias=ln_mg)
                    out_t = p7.tile([P, KC, TO], F32, tag="out")
                    for oc in range(KC):
                        nc.vector.tensor_scalar_mul(out_t[:, oc, :],
                                                    xc1[:, oc, :], c1)
                        tmp = p7s.tile([P, TO], F32, tag="t7")
                        nc.vector.tensor_mul(tmp[:], ym[:, oc, :], msc2[:])
                        nc.vector.tensor_add(out_t[:, oc, :], out_t[:, oc, :],
                                             tmp[:])
                        dma(y_r[:, oc, :], out_t[:, oc, :])
    nc.compile()
    return nc


def kernel(x, c, w_cond, w_qkv, w_out, w_mlp1, w_mlp2, attn_gain, mlp_gain):
    import ml_dtypes
    from concourse.bass_utils import run_bass_kernel_spmd

    (wn_qkv, wn_out, wn_mlp1, wn_mlp2, g1, shift, ag_c, mg_c, c1) = _host_prep(
        x, c, w_cond, w_qkv, w_out, w_mlp1, w_mlp2, attn_gain, mlp_gain)

    wdt = np.float32 if _MMDT_NAME == "f32r" else ml_dtypes.bfloat16
    wn_qkv = np.ascontiguousarray(wn_qkv.astype(wdt))
    wn_out = np.ascontiguousarray(wn_out.astype(wdt))
    wn_mlp1 = np.ascontiguousarray(wn_mlp1.astype(wdt))
    wn_mlp2 = np.ascontiguousarray(wn_mlp2.astype(wdt))
    ones_c = np.ones((P, P), dtype=ml_dtypes.bfloat16)
    bd_c = np.zeros((P, P), dtype=np.float32)
    bd_c[0:DH, 0:DH] = 1.0
    bd_c[DH:P, DH:P] = 1.0
    bd_c = bd_c.astype(ml_dtypes.bfloat16)

    x32 = np.asarray(x, np.float32)
    g1_32 = g1.astype(np.float32)
    sh_32 = shift.astype(np.float32)

    in_maps = []
    for core in range(NCORES):
        b, half = core // 2, core % 2
        xb = x32[b]                                    # [N, HID]
        own = np.ascontiguousarray(xb[half * TO:(half + 1) * TO].T)
        oth = np.ascontiguousarray(xb[(1 - half) * TO:(2 - half) * TO].T)
        gs = np.empty((P, KC, 2), np.float32)
        gs[:, :, 0] = g1_32[b].reshape(KC, P).T
        gs[:, :, 1] = sh_32[b].reshape(KC, P).T
        in_maps.append({
            "x_own": own, "x_oth": oth, "gs": gs,
            "w_qkv": wn_qkv, "w_out": wn_out,
            "w_mlp1": wn_mlp1, "w_mlp2": wn_mlp2,
            "ones_c": ones_c, "bd_c": bd_c,
        })

    nc = _build_nc(ag_c, mg_c, c1)
    res = run_bass_kernel_spmd(nc, in_maps, core_ids=list(range(NCORES)),
                               trace=_TRACE)
    if _TRACE and res.exec_time_ns is not None:
        print(f"HW exec time: {res.exec_time_ns} ns")

    out = np.empty((B, N, HID), np.float32)
    for core in range(NCORES):
        b, half = core // 2, core % 2
        out[b, half * TO:(half + 1) * TO, :] = res.results[core]["y"].T
    kernel._last_results = res
    return out


# revision 12
# speedup vs baseline: 1.1736x; 1.0944x over previous
"""Trainium2 Bass kernel for the DiT block (B=4, N=1024, HID=1024, NH=16, MLP=4096).

Sharding: 8 cores = 4 batches x 2 sequence halves. Each core computes the
full block for its 512 tokens (feature-major layout [hidden, tokens]); K/V
for the other half of its batch are recomputed locally (~14% extra FLOPs,
zero collectives). Host gathers/concats the per-core [1024, 512] outputs.

Matmuls run in float32r (TF32-like, 1 cycle/row for moving dim >= 256).
Weight normalization and the tiny conditioning matmul (0.014% of FLOPs)
are folded on the host in float64.
"""

import math
import os

import numpy as np

B, N, HID, NH = 4, 1024, 1024, 16
DH = HID // NH
MLP = 4 * HID
P = 128
KC = HID // P          # 8 hidden chunks
MC = MLP // P          # 32 mlp chunks
TO = N // 2            # 512 own tokens per core
NCORES = 8

_TRACE = os.environ.get("DIT_TRACE", "0") == "1"
_DBG = os.environ.get("DIT_DEBUG_OUTPUTS", "0") == "1"
_MMDT_NAME = os.environ.get("DIT_MMDT", "f32r")  # f32r | bf16


def _normalize_w(w):
    w = np.asarray(w, np.float64)
    norm = np.linalg.norm(w, axis=1, keepdims=True)
    alpha = math.sqrt(w.shape[1])
    wn = w / (norm * alpha + 1e-4)
    wn = wn / math.sqrt(w.shape[0])
    return wn


def _host_prep(x, c, w_cond, w_qkv, w_out, w_mlp1, w_mlp2, attn_gain, mlp_gain):
    """Float64 host-side folding: weight norms, conditioning, scalars."""
    wn_qkv = _normalize_w(w_qkv)
    wn_out = _normalize_w(w_out)
    wn_mlp1 = _normalize_w(w_mlp1)
    wn_mlp2 = _normalize_w(w_mlp2)
    wn_cond = _normalize_w(w_cond)

    c64 = np.asarray(c, np.float64)
    silu = c64 / (1.0 + np.exp(-c64))
    cc = (silu / 0.596) @ wn_cond              # [B, 2H]
    gain, shift = cc[:, :HID], cc[:, HID:]
    g1 = 1.0 + gain

    ag_c = float(np.exp(np.float64(attn_gain)) * 0.3 / math.sqrt(0.58))
    mg_c = float(np.exp(np.float64(mlp_gain)) * 0.3 / math.sqrt(0.58))
    c1 = float(0.7 / math.sqrt(0.58))
    return (wn_qkv, wn_out, wn_mlp1, wn_mlp2, g1, shift, ag_c, mg_c, c1)


def _build_nc(ag_c, mg_c, c1):
    import concourse.bass as bass
    import concourse.tile as tile
    from concourse import bacc, mybir

    MMDT = mybir.dt.float32r if _MMDT_NAME == "f32r" else mybir.dt.bfloat16
    BF16 = mybir.dt.bfloat16
    F32 = mybir.dt.float32
    AF = mybir.ActivationFunctionType
    ALU = mybir.AluOpType

    import concourse.bacc as _bacc_mod
    from concourse.hw_specs import get_activation_tables as _gat

    def _steered_tables(arch):
        # Keep act_info.json set order (act_func_set_id is positional), but
        # strip our funcs from all sets except the two we want, so the
        # selector binds Copy/Exp/Ln/Square to one set and Silu to another
        # (minimizes ACT table reloads).
        t = dict(_gat(arch))
        keep_all = "natural_log_exp_and_others"
        steer = {f for f in t.get(keep_all, ())
                 if f.name in ("Copy", "Exp", "Ln", "Square", "MemsetZero",
                               "Identity")}
        out = {}
        for name, funcs in t.items():
            if name == keep_all:
                out[name] = funcs
            elif name == "silu_and_others":
                out[name] = {f for f in funcs if f.name not in
                             ("Copy", "Exp", "Ln", "Square")}
            else:
                out[name] = funcs - steer
        return out

    _bacc_mod.get_activation_tables = _steered_tables

    nc = bacc.Bacc()

    # ---- dram I/O ----
    x_own_d = nc.dram_tensor("x_own", [HID, TO], F32, kind="ExternalInput")
    x_oth_d = nc.dram_tensor("x_oth", [HID, TO], F32, kind="ExternalInput")
    gs_d = nc.dram_tensor("gs", [P, KC, 2], F32, kind="ExternalInput")
    wqkv_d = nc.dram_tensor("w_qkv", [HID, 3 * HID], MMDT, kind="ExternalInput")
    wout_d = nc.dram_tensor("w_out", [HID, HID], MMDT, kind="ExternalInput")
    w1_d = nc.dram_tensor("w_mlp1", [HID, MLP], MMDT, kind="ExternalInput")
    w2_d = nc.dram_tensor("w_mlp2", [MLP, HID], MMDT, kind="ExternalInput")
    ones_d = nc.dram_tensor("ones_c", [P, P], BF16, kind="ExternalInput")
    bd_d = nc.dram_tensor("bd_c", [P, P], BF16, kind="ExternalInput")
    y_d = nc.dram_tensor("y", [HID, TO], F32, kind="ExternalOutput")

    dbg = {}
    if _DBG:
        for nm, shp in [("xc", [HID, N]), ("q", [HID, TO]), ("k", [HID, N]),
                        ("ao", [HID, TO]), ("x1", [HID, TO]), ("hmid", [MLP, TO]),
                        ("vtm", [N, HID])]:
            dbg[nm] = nc.dram_tensor("dbg_" + nm, shp, F32, kind="ExternalOutput")

    xo_r = x_own_d.rearrange("(kc ki) t -> ki kc t", ki=P)
    xt_r = x_oth_d.rearrange("(kc ki) t -> ki kc t", ki=P)
    wqkv_r = wqkv_d.rearrange("(kc ki) m -> ki kc m", ki=P)
    wout_r = wout_d.rearrange("(kc ki) m -> ki kc m", ki=P)
    w1_r = w1_d.rearrange("(kc ki) m -> ki kc m", ki=P)
    w2_r = w2_d.rearrange("(kc ki) m -> ki kc m", ki=P)
    y_r = y_d.rearrange("(kc ki) t -> ki kc t", ki=P)

    dma = nc.gpsimd.dma_start

    with tile.TileContext(nc) as tc:
        with tc.tile_pool(name="pp", bufs=1) as pp, \
             tc.tile_pool(name="pdbg", bufs=2) as pdbg, \
             tc.tile_pool(name="ps_acc", bufs=4, space="PSUM") as ps_acc, \
             tc.tile_pool(name="ps_norm", bufs=2, space="PSUM") as ps_norm, \
             tc.tile_pool(name="ps_long", bufs=2, space="PSUM") as ps_long, \
             tc.tile_pool(name="dram", bufs=1, space="DRAM") as drp:

            # ---- persistent small constants ----
            ones_c = pp.tile([P, P], BF16, tag="ones")
            dma(ones_c[:], ones_d[:, :])
            bd_c = pp.tile([P, P], BF16, tag="bd")
            dma(bd_c[:], bd_d[:, :])
            gs_sb = pp.tile([P, KC, 2], F32, tag="gs")
            dma(gs_sb[:], gs_d[:, :, :])
            eps4 = pp.tile([P, 1], F32, tag="eps4")
            nc.vector.memset(eps4[:], 1e-4)
            epsq = pp.tile([P, 1], F32, tag="epsq")
            nc.vector.memset(epsq[:], 1e-10)
            epsm = pp.tile([P, 1], F32, tag="epsm")
            nc.vector.memset(epsm[:], 1e-4 * 0.596 * 0.596)
            onecol = pp.tile([P, 1], F32, tag="onecol")
            nc.vector.memset(onecol[:], 1.0)
            ln_ag = pp.tile([P, 1], F32, tag="lnag")
            nc.vector.memset(ln_ag[:], math.log(ag_c))
            ln_mg = pp.tile([P, 1], F32, tag="lnmg")
            nc.vector.memset(ln_mg[:], math.log(mg_c))

            x_own = pp.tile([P, KC, TO], F32, tag="xown")
            dma(x_own[:], xo_r)

            def pixel_scale(pool, psum, mean_div, eps_tile, tag, lnbias=None):
                """scale = g/sqrt(psum/mean_div + eps) via exp(-0.5*ln(x)+ln(g)),
                ACT-engine only (no slow DVE reciprocal)."""
                s = pool.tile([P, TO], F32, tag=tag)
                nc.scalar.activation(s[:], psum[:], AF.Ln,
                                     scale=1.0 / mean_div, bias=eps_tile[:])
                if lnbias is None:
                    nc.scalar.activation(s[:], s[:], AF.Exp, scale=-0.5)
                else:
                    nc.scalar.activation(s[:], s[:], AF.Exp, scale=-0.5,
                                         bias=lnbias[:])
                return s

            def dbg_dump(name, src, nchunks, cols):
                """Stream src [P, nchunks, cols] (any dtype) to dbg[name]."""
                if not _DBG:
                    return
                dst = dbg[name].rearrange("(kc ki) t -> ki kc t", ki=P)
                for ch in range(nchunks):
                    t = pdbg.tile([P, cols], F32, tag="dbgt")
                    nc.vector.tensor_copy(t[:], src[:, ch, :])
                    dma(dst[:, ch, :], t[:])

            with tc.tile_pool(name="p_ao", bufs=1) as p_ao:
                with tc.tile_pool(name="p_qkv", bufs=1) as p_qkv:

                    # ============ phase 1+2: x_cond, QKV ============
                    with tc.tile_pool(name="p12", bufs=1) as p12, \
                         tc.tile_pool(name="p12w", bufs=2) as p12w, \
                         tc.tile_pool(name="p12v", bufs=1) as p12v, \
                         tc.tile_pool(name="p12s", bufs=2) as p12s:
                        xc = p12.tile([P, KC, N], MMDT, tag="xc")
                        with tc.tile_pool(name="p1x", bufs=1) as p1x:
                            x_oth = p1x.tile([P, KC, TO], F32, tag="xoth")
                            dma(x_oth[:], xt_r)

                            for half, xsrc in ((0, x_own), (1, x_oth)):
                                psn = ps_norm.tile([P, TO], F32, tag="n")
                                for kc in range(KC):
                                    sq = p12s.tile([P, TO], BF16, tag="sq")
                                    nc.vector.tensor_mul(sq[:], xsrc[:, kc, :],
                                                         xsrc[:, kc, :])
                                    nc.tensor.matmul(psn[:], ones_c[:], sq[:],
                                                     start=(kc == 0),
                                                     stop=(kc == KC - 1))
                                sc = pixel_scale(p12s, psn, HID, eps4, "pnsc")
                                for kc in range(KC):
                                    cols = slice(half * TO, half * TO + TO)
                                    nc.vector.tensor_mul(xc[:, kc, cols],
                                                         xsrc[:, kc, :], sc[:])
                                    nc.vector.tensor_scalar(
                                        xc[:, kc, cols], xc[:, kc, cols],
                                        gs_sb[:, kc, 0:1], gs_sb[:, kc, 1:2],
                                        ALU.mult, ALU.add)

                        dbg_dump("xc", xc, KC, N)

                        q_sb = p_qkv.tile([P, KC, TO], BF16, tag="q")
                        k_sb = p_qkv.tile([P, KC, N], BF16, tag="k")
                        v_sb = p_qkv.tile([P, KC, NH, DH + 1], BF16, tag="v")
                        nc.vector.tensor_copy(
                            v_sb[:, :, :, DH:DH + 1],
                            onecol[:].to_broadcast((P, KC, NH, 1)))

                        # ---- Q (+ per-head L2), K ----
                        for proj, out_t, nhalves in (("q", q_sb, 1),
                                                     ("k", k_sb, 2)):
                            off = 0 if proj == "q" else HID
                            for oc in range(KC):
                                wt = p12w.tile([P, KC, P], MMDT, tag="wqkv")
                                dma(wt[:], wqkv_r[:, :,
                                                  off + oc * P: off + (oc + 1) * P])
                                for half in range(nhalves):
                                    cols = slice(half * TO, half * TO + TO)
                                    psA = ps_acc.tile([P, TO], F32, tag="acc")
                                    for kc in range(KC):
                                        nc.tensor.matmul(psA[:], wt[:, kc, :],
                                                         xc[:, kc, cols],
                                                         start=(kc == 0),
                                                         stop=(kc == KC - 1))
                                    sqq = p12s.tile([P, TO], BF16, tag="sqq")
                                    nc.scalar.activation(sqq[:], psA[:],
                                                         AF.Square)
                                    psB = ps_norm.tile([P, TO], F32, tag="n")
                                    nc.tensor.matmul(psB[:], bd_c[:], sqq[:],
                                                     start=True, stop=True)
                                    hsc = p12s.tile([P, TO], F32, tag="hsc")
                                    nc.scalar.activation(hsc[:], psB[:], AF.Ln,
                                                         scale=1.0, bias=epsq[:])
                                    nc.scalar.activation(hsc[:], hsc[:], AF.Exp,
                                                         scale=-0.5)
                                    nc.vector.tensor_mul(out_t[:, oc, cols],
                                                         psA[:], hsc[:])

                        # ---- V (token-major, pixel-normed, ones col) ----
                        vstats = p12s.tile([P, KC, 2, nc.vector.BN_STATS_DIM],
                                           F32, tag="vstats")
                        for vc in range(2):
                            wt = p12v.tile([P, KC, TO], MMDT, tag="wv")
                            dma(wt[:], wqkv_r[:, :, 2 * HID + vc * TO:
                                              2 * HID + (vc + 1) * TO])
                            for tt in range(KC):
                                psV = ps_acc.tile([P, TO], F32, tag="acc")
                                for kc in range(KC):
                                    nc.tensor.matmul(
                                        psV[:], xc[:, kc, tt * P:(tt + 1) * P],
                                        wt[:, kc, :],
                                        start=(kc == 0), stop=(kc == KC - 1))
                                nc.vector.bn_stats(vstats[:, tt, vc, :], psV[:])
                                nc.scalar.copy(
                                    v_sb[:, tt, vc * 8:(vc + 1) * 8, 0:DH],
                                    psV[:].rearrange("p (h d) -> p h d", d=DH))
                        for tt in range(KC):
                            mv = p12s.tile([P, nc.vector.BN_AGGR_DIM], F32,
                                           tag="mv")
                            nc.vector.bn_aggr(mv[:], vstats[:, tt, :, :])
                            m2 = p12s.tile([P, 1], F32, tag="m2")
                            nc.vector.tensor_mul(m2[:], mv[:, 0:1], mv[:, 0:1])
                            nc.vector.tensor_add(m2[:], m2[:], mv[:, 1:2])
                            nc.scalar.activation(m2[:], m2[:], AF.Ln,
                                                 scale=1.0, bias=eps4[:])
                            nc.scalar.activation(m2[:], m2[:], AF.Exp,
                                                 scale=-0.5)
                            nc.vector.tensor_scalar_mul(
                                v_sb[:, tt, :, 0:DH], v_sb[:, tt, :, 0:DH],
                                m2[:])

                        dbg_dump("q", q_sb, KC, TO)
                        dbg_dump("k", k_sb, KC, N)
                        if _DBG:
                            vdst = dbg["vtm"].rearrange(
                                "(tt ki) d -> ki tt d", ki=P)
                            for tt in range(KC):
                                t = pdbg.tile([P, NH, DH], F32, tag="dbgv")
                                nc.vector.tensor_copy(t[:],
                                                      v_sb[:, tt, :, 0:DH])
                                dma(vdst[:, tt, :],
                                    t[:].rearrange("p h d -> p (h d)"))

                    # ============ phase 3: attention ============
                    ao = p_ao.tile([P, KC, TO], MMDT, tag="ao")
                    with tc.tile_pool(name="p3", bufs=1) as p3, \
                         tc.tile_pool(name="p3r", bufs=2) as p3r, \
                         tc.tile_pool(name="p3q", bufs=4) as p3q:
                        r_dram = drp.tile([NH, TO], F32, tag="rd")
                        for hp in range(NH // 2):
                            hc = hp
                            pTs = []
                            for sub in range(2):
                                po = DH * sub
                                pT = p3r.tile([P, KC, TO], BF16, tag="pT")
                                pTs.append(pT)
                            # interleave the two heads' score MMs: they sit in
                            # disjoint row groups (partitions 0-63 / 64-127) so
                            # the PE array runs them concurrently
                            for kt in range(KC):
                                for sub in range(2):
                                    po = DH * sub
                                    ps_s = ps_acc.tile([P, TO], F32, tag="acc")
                                    nc.tensor.matmul(
                                        ps_s[:],
                                        k_sb[po:po + DH, hc,
                                             kt * P:(kt + 1) * P],
                                        q_sb[po:po + DH, hc, :],
                                        start=True, stop=True)
                                    nc.scalar.activation(pTs[sub][:, kt, :],
                                                         ps_s[:], AF.Exp,
                                                         scale=0.125)
                            for sub in range(2):
                                h = 2 * hp + sub
                                po = DH * sub
                                pT = pTs[sub]
                                ps_av = ps_long.tile([DH + 1, TO], F32,
                                                     tag="av")
                                for kt in range(KC):
                                    nc.tensor.matmul(ps_av[:],
                                                     v_sb[:, kt, h, :],
                                                     pT[:, kt, :],
                                                     start=(kt == 0),
                                                     stop=(kt == KC - 1))
                                # 1/r for this head straight from psum row 64
                                rext = p3q.tile([DH + 1, TO], F32, tag="rext")
                                nc.scalar.activation(rext[DH:DH + 1, :],
                                                     ps_av[DH:DH + 1, :],
                                                     AF.Ln)
                                nc.scalar.activation(rext[DH:DH + 1, :],
                                                     rext[DH:DH + 1, :],
                                                     AF.Exp, scale=-1.0)
                                dma(r_dram[h:h + 1, :], rext[DH:DH + 1, :])
                                rrh = p3q.tile([DH, TO], F32, tag="rrh")
                                src = bass.AP(
                                    tensor=r_dram.tensor,
                                    offset=r_dram.offset + h * TO,
                                    ap=[[0, DH], [1, TO]])
                                dma(rrh[:], src)
                                stage = p3q.tile([DH, TO], MMDT, tag="stage")
                                nc.vector.tensor_mul(stage[:], ps_av[0:DH, :],
                                                     rrh[:])
                                dma(ao[po:po + DH, hc, :], stage[:])
                        dbg_dump("ao", ao, KC, TO)

                # ============ phase 4: out-proj + residual ============
                with tc.tile_pool(name="p4", bufs=1) as p4, \
                     tc.tile_pool(name="p4w", bufs=3) as p4w, \
                     tc.tile_pool(name="p4s", bufs=2) as p4s:
                    yo = p4.tile([P, KC, TO], F32, tag="yo")
                    xc1 = pp.tile([P, KC, TO], F32, tag="x1")
                    psn = ps_long.tile([P, TO], F32, tag="av")
                    for oc in range(KC):
                        wt = p4w.tile([P, KC, P], MMDT, tag="wo")
                        dma(wt[:], wout_r[:, :, oc * P:(oc + 1) * P])
                        psA = ps_acc.tile([P, TO], F32, tag="acc")
                        for kc in range(KC):
                            nc.tensor.matmul(psA[:], wt[:, kc, :], ao[:, kc, :],
                                             start=(kc == 0), stop=(kc == KC - 1))
                        nc.scalar.copy(yo[:, oc, :], psA[:])
                        sqo = p4s.tile([P, TO], BF16, tag="sqo")
                        nc.vector.tensor_mul(sqo[:], yo[:, oc, :], yo[:, oc, :])
                        nc.tensor.matmul(psn[:], ones_c[:], sqo[:],
                                         start=(oc == 0), stop=(oc == KC - 1))
                    osc = pixel_scale(p4s, psn, HID, eps4, "osc", lnbias=ln_ag)
                    for oc in range(KC):
                        nc.vector.tensor_scalar_mul(xc1[:, oc, :],
                                                    x_own[:, oc, :], c1)
                        tmp = p4s.tile([P, TO], F32, tag="t4")
                        nc.vector.tensor_mul(tmp[:], yo[:, oc, :], osc[:])
                        nc.vector.tensor_add(xc1[:, oc, :], xc1[:, oc, :],
                                             tmp[:])
                    dbg_dump("x1", xc1, KC, TO)

            # ============ phase 5: x_cond2 ============
            with tc.tile_pool(name="p5", bufs=2) as p5:
                xc2 = pp.tile([P, KC, TO], MMDT, tag="xc2")
                psn = ps_norm.tile([P, TO], F32, tag="n")
                for kc in range(KC):
                    sq = p5.tile([P, TO], BF16, tag="sq5")
                    nc.vector.tensor_mul(sq[:], xc1[:, kc, :], xc1[:, kc, :])
                    nc.tensor.matmul(psn[:], ones_c[:], sq[:],
                                     start=(kc == 0), stop=(kc == KC - 1))
                sc = pixel_scale(p5, psn, HID, eps4, "sc5")
                for kc in range(KC):
                    nc.vector.tensor_mul(xc2[:, kc, :], xc1[:, kc, :], sc[:])
                    nc.vector.tensor_scalar(xc2[:, kc, :], xc2[:, kc, :],
                                            gs_sb[:, kc, 0:1], gs_sb[:, kc, 1:2],
                                            ALU.mult, ALU.add)

            # ============ phase 6+7: MLP ============
            with tc.tile_pool(name="p67", bufs=1) as p67:
                h_sb = p67.tile([P, MC, TO], MMDT, tag="h")
                with tc.tile_pool(name="p6w", bufs=3) as p6w, \
                     tc.tile_pool(name="p6s", bufs=2) as p6s:
                    psn = ps_long.tile([P, TO], F32, tag="av")
                    for oc in range(MC):
                        wt = p6w.tile([P, KC, P], MMDT, tag="w1")
                        dma(wt[:], w1_r[:, :, oc * P:(oc + 1) * P])
                        psA = ps_acc.tile([P, TO], F32, tag="acc")
                        for kc in range(KC):
                            nc.tensor.matmul(psA[:], wt[:, kc, :], xc2[:, kc, :],
                                             start=(kc == 0), stop=(kc == KC - 1))
                        nc.scalar.copy(h_sb[:, oc, :], psA[:])
                        sqh = p6s.tile([P, TO], BF16, tag="sqh")
                        nc.vector.tensor_mul(sqh[:], h_sb[:, oc, :],
                                             h_sb[:, oc, :])
                        nc.tensor.matmul(psn[:], ones_c[:], sqh[:],
                                         start=(oc == 0), stop=(oc == MC - 1))
                    msc = pixel_scale(p6s, psn, MLP, eps4, "msc")
                    for oc in range(MC):
                        nc.vector.tensor_mul(h_sb[:, oc, :], h_sb[:, oc, :],
                                             msc[:])
                        nc.scalar.activation(h_sb[:, oc, :], h_sb[:, oc, :],
                                             AF.Silu)
                    dbg_dump("hmid", h_sb, MC, TO)

                with tc.tile_pool(name="p7", bufs=1) as p7, \
                     tc.tile_pool(name="p7w", bufs=2) as p7w, \
                     tc.tile_pool(name="p7s", bufs=2) as p7s:
                    ym = p7.tile([P, KC, TO], F32, tag="ym")
                    psn = ps_long.tile([P, TO], F32, tag="av")
                    for oc in range(KC):
                        wt = p7w.tile([P, MC, P], MMDT, tag="w2")
                        dma(wt[:], w2_r[:, :, oc * P:(oc + 1) * P])
                        psA = ps_acc.tile([P, TO], F32, tag="acc")
                        for kc in range(MC):
                            nc.tensor.matmul(psA[:], wt[:, kc, :], h_sb[:, kc, :],
                                             start=(kc == 0), stop=(kc == MC - 1))
                        nc.scalar.copy(ym[:, oc, :], psA[:])
                        sqm = p7s.tile([P, TO], BF16, tag="sqm")
                        nc.vector.tensor_mul(sqm[:], ym[:, oc, :], ym[:, oc, :])
                        nc.tensor.matmul(psn[:], ones_c[:], sqm[:],
                                         start=(oc == 0), stop=(oc == KC - 1))
                    msc2 = pixel_scale(p7s, psn, HID, epsm, "msc2",
                                       lnbias=ln_mg)
                    out_t = p7.tile([P, KC, TO], F32, tag="out")
                    for oc in range(KC):
                        nc.vector.tensor_scalar_mul(out_t[:, oc, :],
                                                    xc1[:, oc, :], c1)
                        tmp = p7s.tile([P, TO], F32, tag="t7")
                        nc.vector.tensor_mul(tmp[:], ym[:, oc, :], msc2[:])
                        nc.vector.tensor_add(out_t[:, oc, :], out_t[:, oc, :],
                                             tmp[:])
                        dma(y_r[:, oc, :], out_t[:, oc, :])
    nc.compile()
    return nc


def kernel(x, c, w_cond, w_qkv, w_out, w_mlp1, w_mlp2, attn_gain, mlp_gain):
    import ml_dtypes
    from concourse.bass_utils import run_bass_kernel_spmd

    (wn_qkv, wn_out, wn_mlp1, wn_mlp2, g1, shift, ag_c, mg_c, c1) = _host_prep(
        x, c, w_cond, w_qkv, w_out, w_mlp1, w_mlp2, attn_gain, mlp_gain)

    wdt = np.float32 if _MMDT_NAME == "f32r" else ml_dtypes.bfloat16
    wn_qkv = np.ascontiguousarray(wn_qkv.astype(wdt))
    wn_out = np.ascontiguousarray(wn_out.astype(wdt))
    wn_mlp1 = np.ascontiguousarray(wn_mlp1.astype(wdt))
    wn_mlp2 = np.ascontiguousarray(wn_mlp2.astype(wdt))
    ones_c = np.ones((P, P), dtype=ml_dtypes.bfloat16)
    bd_c = np.zeros((P, P), dtype=np.float32)
    bd_c[0:DH, 0:DH] = 1.0
    bd_c[DH:P, DH:P] = 1.0
    bd_c = bd_c.astype(ml_dtypes.bfloat16)

    x32 = np.asarray(x, np.float32)
    g1_32 = g1.astype(np.float32)
    sh_32 = shift.astype(np.float32)

    in_maps = []
    for core in range(NCORES):
        b, half = core // 2, core % 2
        xb = x32[b]                                    # [N, HID]
        own = np.ascontiguousarray(xb[half * TO:(half + 1) * TO].T)
        oth = np.ascontiguousarray(xb[(1 - half) * TO:(2 - half) * TO].T)
        gs = np.empty((P, KC, 2), np.float32)
        gs[:, :, 0] = g1_32[b].reshape(KC, P).T
        gs[:, :, 1] = sh_32[b].reshape(KC, P).T
        in_maps.append({
            "x_own": own, "x_oth": oth, "gs": gs,
            "w_qkv": wn_qkv, "w_out": wn_out,
            "w_mlp1": wn_mlp1, "w_mlp2": wn_mlp2,
            "ones_c": ones_c, "bd_c": bd_c,
        })

    nc = _build_nc(ag_c, mg_c, c1)
    res = run_bass_kernel_spmd(nc, in_maps, core_ids=list(range(NCORES)),
                               trace=_TRACE)
    if _TRACE and res.exec_time_ns is not None:
        print(f"HW exec time: {res.exec_time_ns} ns")

    out = np.empty((B, N, HID), np.float32)
    for core in range(NCORES):
        b, half = core // 2, core % 2
        out[b, half * TO:(half + 1) * TO, :] = res.results[core]["y"].T
    kernel._last_results = res
    return out


# revision 13
# speedup vs baseline: 1.2538x; 1.0684x over previous
"""Trainium2 Bass kernel for the DiT block (B=4, N=1024, HID=1024, NH=16, MLP=4096).

Sharding: 8 cores = 4 batches x 2 sequence halves. Each core computes the
full block for its 512 tokens (feature-major layout [hidden, tokens]); K/V
for the other half of its batch are recomputed locally (~14% extra FLOPs,
zero collectives). Host gathers/concats the per-core [1024, 512] outputs.

Matmuls run in float32r (TF32-like, 1 cycle/row for moving dim >= 256).
Weight normalization and the tiny conditioning matmul (0.014% of FLOPs)
are folded on the host in float64.
"""

import math
import os

import numpy as np

B, N, HID, NH = 4, 1024, 1024, 16
DH = HID // NH
MLP = 4 * HID
P = 128
KC = HID // P          # 8 hidden chunks
MC = MLP // P          # 32 mlp chunks
TO = N // 2            # 512 own tokens per core
NCORES = 8

_TRACE = os.environ.get("DIT_TRACE", "0") == "1"
_DBG = os.environ.get("DIT_DEBUG_OUTPUTS", "0") == "1"
_MMDT_NAME = os.environ.get("DIT_MMDT", "f32r")  # f32r | bf16


def _normalize_w(w):
    w = np.asarray(w, np.float64)
    norm = np.linalg.norm(w, axis=1, keepdims=True)
    alpha = math.sqrt(w.shape[1])
    wn = w / (norm * alpha + 1e-4)
    wn = wn / math.sqrt(w.shape[0])
    return wn


def _host_prep(x, c, w_cond, w_qkv, w_out, w_mlp1, w_mlp2, attn_gain, mlp_gain):
    """Float64 host-side folding: weight norms, conditioning, scalars."""
    wn_qkv = _normalize_w(w_qkv)
    wn_out = _normalize_w(w_out)
    wn_mlp1 = _normalize_w(w_mlp1)
    wn_mlp2 = _normalize_w(w_mlp2)
    wn_cond = _normalize_w(w_cond)

    c64 = np.asarray(c, np.float64)
    silu = c64 / (1.0 + np.exp(-c64))
    cc = (silu / 0.596) @ wn_cond              # [B, 2H]
    gain, shift = cc[:, :HID], cc[:, HID:]
    g1 = 1.0 + gain

    ag_c = float(np.exp(np.float64(attn_gain)) * 0.3 / math.sqrt(0.58))
    mg_c = float(np.exp(np.float64(mlp_gain)) * 0.3 / math.sqrt(0.58))
    c1 = float(0.7 / math.sqrt(0.58))
    return (wn_qkv, wn_out, wn_mlp1, wn_mlp2, g1, shift, ag_c, mg_c, c1)


def _build_nc(ag_c, mg_c, c1):
    import concourse.bass as bass
    import concourse.tile as tile
    from concourse import bacc, mybir

    MMDT = mybir.dt.float32r if _MMDT_NAME == "f32r" else mybir.dt.bfloat16
    BF16 = mybir.dt.bfloat16
    F32 = mybir.dt.float32
    AF = mybir.ActivationFunctionType
    ALU = mybir.AluOpType

    import concourse.bacc as _bacc_mod
    from concourse.hw_specs import get_activation_tables as _gat

    def _steered_tables(arch):
        # Keep act_info.json set order (act_func_set_id is positional), but
        # strip our funcs from all sets except the two we want, so the
        # selector binds Copy/Exp/Ln/Square to one set and Silu to another
        # (minimizes ACT table reloads).
        t = dict(_gat(arch))
        keep_all = "natural_log_exp_and_others"
        steer = {f for f in t.get(keep_all, ())
                 if f.name in ("Copy", "Exp", "Ln", "Square", "MemsetZero",
                               "Identity")}
        out = {}
        for name, funcs in t.items():
            if name == keep_all:
                out[name] = funcs
            elif name == "silu_and_others":
                out[name] = {f for f in funcs if f.name not in
                             ("Copy", "Exp", "Ln", "Square")}
            else:
                out[name] = funcs - steer
        return out

    _bacc_mod.get_activation_tables = _steered_tables

    nc = bacc.Bacc()

    # ---- dram I/O ----
    x_own_d = nc.dram_tensor("x_own", [HID, TO], F32, kind="ExternalInput")
    x_oth_d = nc.dram_tensor("x_oth", [HID, TO], F32, kind="ExternalInput")
    gs_d = nc.dram_tensor("gs", [P, KC, 2], F32, kind="ExternalInput")
    wqkv_d = nc.dram_tensor("w_qkv", [HID, 3 * HID], MMDT, kind="ExternalInput")
    wout_d = nc.dram_tensor("w_out", [HID, HID], MMDT, kind="ExternalInput")
    w1_d = nc.dram_tensor("w_mlp1", [HID, MLP], MMDT, kind="ExternalInput")
    w2_d = nc.dram_tensor("w_mlp2", [MLP, HID], MMDT, kind="ExternalInput")
    ones_d = nc.dram_tensor("ones_c", [P, P], BF16, kind="ExternalInput")
    bd_d = nc.dram_tensor("bd_c", [P, P], BF16, kind="ExternalInput")
    y_d = nc.dram_tensor("y", [HID, TO], F32, kind="ExternalOutput")

    dbg = {}
    if _DBG:
        for nm, shp in [("xc", [HID, N]), ("q", [HID, TO]), ("k", [HID, N]),
                        ("ao", [HID, TO]), ("x1", [HID, TO]), ("hmid", [MLP, TO]),
                        ("vtm", [N, HID])]:
            dbg[nm] = nc.dram_tensor("dbg_" + nm, shp, F32, kind="ExternalOutput")

    xo_r = x_own_d.rearrange("(kc ki) t -> ki kc t", ki=P)
    xt_r = x_oth_d.rearrange("(kc ki) t -> ki kc t", ki=P)
    wqkv_r = wqkv_d.rearrange("(kc ki) m -> ki kc m", ki=P)
    wout_r = wout_d.rearrange("(kc ki) m -> ki kc m", ki=P)
    w1_r = w1_d.rearrange("(kc ki) m -> ki kc m", ki=P)
    w2_r = w2_d.rearrange("(kc ki) m -> ki kc m", ki=P)
    y_r = y_d.rearrange("(kc ki) t -> ki kc t", ki=P)

    dma = nc.sync.dma_start

    with tile.TileContext(nc) as tc:
        with tc.tile_pool(name="pp", bufs=1) as pp, \
             tc.tile_pool(name="pdbg", bufs=2) as pdbg, \
             tc.tile_pool(name="ps_acc", bufs=4, space="PSUM") as ps_acc, \
             tc.tile_pool(name="ps_norm", bufs=2, space="PSUM") as ps_norm, \
             tc.tile_pool(name="ps_long", bufs=2, space="PSUM") as ps_long, \
             tc.tile_pool(name="dram", bufs=1, space="DRAM") as drp:

            # ---- persistent small constants ----
            ones_c = pp.tile([P, P], BF16, tag="ones")
            dma(ones_c[:], ones_d[:, :])
            bd_c = pp.tile([P, P], BF16, tag="bd")
            dma(bd_c[:], bd_d[:, :])
            gs_sb = pp.tile([P, KC, 2], F32, tag="gs")
            dma(gs_sb[:], gs_d[:, :, :])
            eps4 = pp.tile([P, 1], F32, tag="eps4")
            nc.vector.memset(eps4[:], 1e-4)
            epsq = pp.tile([P, 1], F32, tag="epsq")
            nc.vector.memset(epsq[:], 1e-10)
            epsm = pp.tile([P, 1], F32, tag="epsm")
            nc.vector.memset(epsm[:], 1e-4 * 0.596 * 0.596)
            onecol = pp.tile([P, 1], F32, tag="onecol")
            nc.vector.memset(onecol[:], 1.0)
            ln_ag = pp.tile([P, 1], F32, tag="lnag")
            nc.vector.memset(ln_ag[:], math.log(ag_c))
            ln_mg = pp.tile([P, 1], F32, tag="lnmg")
            nc.vector.memset(ln_mg[:], math.log(mg_c))

            x_own = pp.tile([P, KC, TO], F32, tag="xown")
            dma(x_own[:], xo_r)

            def pixel_scale(pool, psum, mean_div, eps_tile, tag, lnbias=None):
                """scale = g/sqrt(psum/mean_div + eps) via exp(-0.5*ln(x)+ln(g)),
                ACT-engine only (no slow DVE reciprocal)."""
                s = pool.tile([P, TO], F32, tag=tag)
                nc.scalar.activation(s[:], psum[:], AF.Ln,
                                     scale=1.0 / mean_div, bias=eps_tile[:])
                if lnbias is None:
                    nc.scalar.activation(s[:], s[:], AF.Exp, scale=-0.5)
                else:
                    nc.scalar.activation(s[:], s[:], AF.Exp, scale=-0.5,
                                         bias=lnbias[:])
                return s

            def dbg_dump(name, src, nchunks, cols):
                """Stream src [P, nchunks, cols] (any dtype) to dbg[name]."""
                if not _DBG:
                    return
                dst = dbg[name].rearrange("(kc ki) t -> ki kc t", ki=P)
                for ch in range(nchunks):
                    t = pdbg.tile([P, cols], F32, tag="dbgt")
                    nc.vector.tensor_copy(t[:], src[:, ch, :])
                    dma(dst[:, ch, :], t[:])

            with tc.tile_pool(name="p_ao", bufs=1) as p_ao:
                with tc.tile_pool(name="p_qkv", bufs=1) as p_qkv:

                    # ============ phase 1+2: x_cond, QKV ============
                    with tc.tile_pool(name="p12", bufs=1) as p12, \
                         tc.tile_pool(name="p12w", bufs=2) as p12w, \
                         tc.tile_pool(name="p12v", bufs=1) as p12v, \
                         tc.tile_pool(name="p12s", bufs=2) as p12s:
                        xc = p12.tile([P, KC, N], MMDT, tag="xc")
                        with tc.tile_pool(name="p1x", bufs=1) as p1x:
                            x_oth = p1x.tile([P, KC, TO], F32, tag="xoth")
                            dma(x_oth[:], xt_r)

                            for half, xsrc in ((0, x_own), (1, x_oth)):
                                psn = ps_norm.tile([P, TO], F32, tag="n")
                                for kc in range(KC):
                                    sq = p12s.tile([P, TO], BF16, tag="sq")
                                    nc.vector.tensor_mul(sq[:], xsrc[:, kc, :],
                                                         xsrc[:, kc, :])
                                    nc.tensor.matmul(psn[:], ones_c[:], sq[:],
                                                     start=(kc == 0),
                                                     stop=(kc == KC - 1))
                                sc = pixel_scale(p12s, psn, HID, eps4, "pnsc")
                                for kc in range(KC):
                                    cols = slice(half * TO, half * TO + TO)
                                    nc.vector.tensor_mul(xc[:, kc, cols],
                                                         xsrc[:, kc, :], sc[:])
                                    nc.vector.tensor_scalar(
                                        xc[:, kc, cols], xc[:, kc, cols],
                                        gs_sb[:, kc, 0:1], gs_sb[:, kc, 1:2],
                                        ALU.mult, ALU.add)

                        dbg_dump("xc", xc, KC, N)

                        q_sb = p_qkv.tile([P, KC, TO], BF16, tag="q")
                        k_sb = p_qkv.tile([P, KC, N], BF16, tag="k")
                        v_sb = p_qkv.tile([P, KC, NH, DH + 1], BF16, tag="v")
                        nc.vector.tensor_copy(
                            v_sb[:, :, :, DH:DH + 1],
                            onecol[:].to_broadcast((P, KC, NH, 1)))

                        # ---- Q (+ per-head L2), K ----
                        for proj, out_t, nhalves in (("q", q_sb, 1),
                                                     ("k", k_sb, 2)):
                            off = 0 if proj == "q" else HID
                            for oc in range(KC):
                                wt = p12w.tile([P, KC, P], MMDT, tag="wqkv")
                                dma(wt[:], wqkv_r[:, :,
                                                  off + oc * P: off + (oc + 1) * P])
                                for half in range(nhalves):
                                    cols = slice(half * TO, half * TO + TO)
                                    psA = ps_acc.tile([P, TO], F32, tag="acc")
                                    for kc in range(KC):
                                        nc.tensor.matmul(psA[:], wt[:, kc, :],
                                                         xc[:, kc, cols],
                                                         start=(kc == 0),
                                                         stop=(kc == KC - 1))
                                    sqq = p12s.tile([P, TO], BF16, tag="sqq")
                                    nc.scalar.activation(sqq[:], psA[:],
                                                         AF.Square)
                                    psB = ps_norm.tile([P, TO], F32, tag="n")
                                    nc.tensor.matmul(psB[:], bd_c[:], sqq[:],
                                                     start=True, stop=True)
                                    hsc = p12s.tile([P, TO], F32, tag="hsc")
                                    nc.scalar.activation(hsc[:], psB[:], AF.Ln,
                                                         scale=1.0, bias=epsq[:])
                                    nc.scalar.activation(hsc[:], hsc[:], AF.Exp,
                                                         scale=-0.5)
                                    nc.vector.tensor_mul(out_t[:, oc, cols],
                                                         psA[:], hsc[:])

                        # ---- V (token-major, pixel-normed, ones col) ----
                        vstats = p12s.tile([P, KC, 2, nc.vector.BN_STATS_DIM],
                                           F32, tag="vstats")
                        for vc in range(2):
                            wt = p12v.tile([P, KC, TO], MMDT, tag="wv")
                            dma(wt[:], wqkv_r[:, :, 2 * HID + vc * TO:
                                              2 * HID + (vc + 1) * TO])
                            for tt in range(KC):
                                psV = ps_acc.tile([P, TO], F32, tag="acc")
                                for kc in range(KC):
                                    nc.tensor.matmul(
                                        psV[:], xc[:, kc, tt * P:(tt + 1) * P],
                                        wt[:, kc, :],
                                        start=(kc == 0), stop=(kc == KC - 1))
                                nc.vector.bn_stats(vstats[:, tt, vc, :], psV[:])
                                nc.scalar.copy(
                                    v_sb[:, tt, vc * 8:(vc + 1) * 8, 0:DH],
                                    psV[:].rearrange("p (h d) -> p h d", d=DH))
                        for tt in range(KC):
                            mv = p12s.tile([P, nc.vector.BN_AGGR_DIM], F32,
                                           tag="mv")
                            nc.vector.bn_aggr(mv[:], vstats[:, tt, :, :])
                            m2 = p12s.tile([P, 1], F32, tag="m2")
                            nc.vector.tensor_mul(m2[:], mv[:, 0:1], mv[:, 0:1])
                            nc.vector.tensor_add(m2[:], m2[:], mv[:, 1:2])
                            nc.scalar.activation(m2[:], m2[:], AF.Ln,
                                                 scale=1.0, bias=eps4[:])
                            nc.scalar.activation(m2[:], m2[:], AF.Exp,
                                                 scale=-0.5)
                            nc.vector.tensor_scalar_mul(
                                v_sb[:, tt, :, 0:DH], v_sb[:, tt, :, 0:DH],
                                m2[:])

                        dbg_dump("q", q_sb, KC, TO)
                        dbg_dump("k", k_sb, KC, N)
                        if _DBG:
                            vdst = dbg["vtm"].rearrange(
                                "(tt ki) d -> ki tt d", ki=P)
                            for tt in range(KC):
                                t = pdbg.tile([P, NH, DH], F32, tag="dbgv")
                                nc.vector.tensor_copy(t[:],
                                                      v_sb[:, tt, :, 0:DH])
                                dma(vdst[:, tt, :],
                                    t[:].rearrange("p h d -> p (h d)"))

                    # ============ phase 3: attention ============
                    ao = p_ao.tile([P, KC, TO], MMDT, tag="ao")
                    with tc.tile_pool(name="p3", bufs=1) as p3, \
                         tc.tile_pool(name="p3r", bufs=2) as p3r, \
                         tc.tile_pool(name="p3q", bufs=4) as p3q:
                        r_dram = drp.tile([NH, TO], F32, tag="rd")
                        for hp in range(NH // 2):
                            hc = hp
                            pTs = []
                            for sub in range(2):
                                po = DH * sub
                                pT = p3r.tile([P, KC, TO], BF16, tag="pT")
                                pTs.append(pT)
                            # interleave the two heads' score MMs: they sit in
                            # disjoint row groups (partitions 0-63 / 64-127) so
                            # the PE array runs them concurrently
                            for kt in range(KC):
                                for sub in range(2):
                                    po = DH * sub
                                    psp = ps_acc if (2 * kt + sub) % 3 else ps_norm
                                    ps_s = psp.tile([P, TO], F32,
                                                    tag="acc" if psp is ps_acc
                                                    else "n")
                                    nc.tensor.matmul(
                                        ps_s[:],
                                        k_sb[po:po + DH, hc,
                                             kt * P:(kt + 1) * P],
                                        q_sb[po:po + DH, hc, :],
                                        start=True, stop=True)
                                    nc.scalar.activation(pTs[sub][:, kt, :],
                                                         ps_s[:], AF.Exp,
                                                         scale=0.125)
                            for sub in range(2):
                                h = 2 * hp + sub
                                po = DH * sub
                                pT = pTs[sub]
                                ps_av = ps_long.tile([DH + 1, TO], F32,
                                                     tag="av")
                                for kt in range(KC):
                                    nc.tensor.matmul(ps_av[:],
                                                     v_sb[:, kt, h, :],
                                                     pT[:, kt, :],
                                                     start=(kt == 0),
                                                     stop=(kt == KC - 1))
                                # 1/r for this head straight from psum row 64
                                rext = p3q.tile([DH + 1, TO], F32, tag="rext")
                                nc.scalar.activation(rext[DH:DH + 1, :],
                                                     ps_av[DH:DH + 1, :],
                                                     AF.Ln)
                                nc.scalar.activation(rext[DH:DH + 1, :],
                                                     rext[DH:DH + 1, :],
                                                     AF.Exp, scale=-1.0)
                                dma(r_dram[h:h + 1, :], rext[DH:DH + 1, :])
                                rrh = p3q.tile([DH, TO], F32, tag="rrh")
                                src = bass.AP(
                                    tensor=r_dram.tensor,
                                    offset=r_dram.offset + h * TO,
                                    ap=[[0, DH], [1, TO]])
                                dma(rrh[:], src)
                                stage = p3q.tile([DH, TO], MMDT, tag="stage")
                                nc.vector.tensor_mul(stage[:], ps_av[0:DH, :],
                                                     rrh[:])
                                dma(ao[po:po + DH, hc, :], stage[:])
                        dbg_dump("ao", ao, KC, TO)

                # ============ phase 4: out-proj + residual ============
                with tc.tile_pool(name="p4", bufs=1) as p4, \
                     tc.tile_pool(name="p4w", bufs=3) as p4w, \
                     tc.tile_pool(name="p4s", bufs=2) as p4s:
                    yo = p4.tile([P, KC, TO], F32, tag="yo")
                    xc1 = pp.tile([P, KC, TO], F32, tag="x1")
                    psn = ps_long.tile([P, TO], F32, tag="av")
                    for oc in range(KC):
                        wt = p4w.tile([P, KC, P], MMDT, tag="wo")
                        dma(wt[:], wout_r[:, :, oc * P:(oc + 1) * P])
                        psA = ps_acc.tile([P, TO], F32, tag="acc")
                        for kc in range(KC):
                            nc.tensor.matmul(psA[:], wt[:, kc, :], ao[:, kc, :],
                                             start=(kc == 0), stop=(kc == KC - 1))
                        nc.scalar.copy(yo[:, oc, :], psA[:])
                        sqo = p4s.tile([P, TO], BF16, tag="sqo")
                        nc.vector.tensor_mul(sqo[:], yo[:, oc, :], yo[:, oc, :])
                        nc.tensor.matmul(psn[:], ones_c[:], sqo[:],
                                         start=(oc == 0), stop=(oc == KC - 1))
                    osc = pixel_scale(p4s, psn, HID, eps4, "osc", lnbias=ln_ag)
                    for oc in range(KC):
                        nc.vector.tensor_scalar_mul(xc1[:, oc, :],
                                                    x_own[:, oc, :], c1)
                        tmp = p4s.tile([P, TO], F32, tag="t4")
                        nc.vector.tensor_mul(tmp[:], yo[:, oc, :], osc[:])
                        nc.vector.tensor_add(xc1[:, oc, :], xc1[:, oc, :],
                                             tmp[:])
                    dbg_dump("x1", xc1, KC, TO)

            # ============ phase 5: x_cond2 ============
            with tc.tile_pool(name="p5", bufs=2) as p5:
                xc2 = pp.tile([P, KC, TO], MMDT, tag="xc2")
                psn = ps_norm.tile([P, TO], F32, tag="n")
                for kc in range(KC):
                    sq = p5.tile([P, TO], BF16, tag="sq5")
                    nc.vector.tensor_mul(sq[:], xc1[:, kc, :], xc1[:, kc, :])
                    nc.tensor.matmul(psn[:], ones_c[:], sq[:],
                                     start=(kc == 0), stop=(kc == KC - 1))
                sc = pixel_scale(p5, psn, HID, eps4, "sc5")
                for kc in range(KC):
                    nc.vector.tensor_mul(xc2[:, kc, :], xc1[:, kc, :], sc[:])
                    nc.vector.tensor_scalar(xc2[:, kc, :], xc2[:, kc, :],
                                            gs_sb[:, kc, 0:1], gs_sb[:, kc, 1:2],
                                            ALU.mult, ALU.add)

            # ============ phase 6+7: MLP ============
            with tc.tile_pool(name="p67", bufs=1) as p67:
                h_sb = p67.tile([P, MC, TO], MMDT, tag="h")
                with tc.tile_pool(name="p6w", bufs=3) as p6w, \
                     tc.tile_pool(name="p6s", bufs=2) as p6s:
                    psn = ps_long.tile([P, TO], F32, tag="av")
                    for oc in range(MC):
                        wt = p6w.tile([P, KC, P], MMDT, tag="w1")
                        dma(wt[:], w1_r[:, :, oc * P:(oc + 1) * P])
                        psA = ps_acc.tile([P, TO], F32, tag="acc")
                        for kc in range(KC):
                            nc.tensor.matmul(psA[:], wt[:, kc, :], xc2[:, kc, :],
                                             start=(kc == 0), stop=(kc == KC - 1))
                        nc.scalar.copy(h_sb[:, oc, :], psA[:])
                        sqh = p6s.tile([P, TO], BF16, tag="sqh")
                        nc.vector.tensor_mul(sqh[:], h_sb[:, oc, :],
                                             h_sb[:, oc, :])
                        nc.tensor.matmul(psn[:], ones_c[:], sqh[:],
                                         start=(oc == 0), stop=(oc == MC - 1))
                    msc = pixel_scale(p6s, psn, MLP, eps4, "msc")
                    for oc in range(MC):
                        nc.vector.tensor_mul(h_sb[:, oc, :], h_sb[:, oc, :],
                                             msc[:])
                        nc.scalar.activation(h_sb[:, oc, :], h_sb[:, oc, :],
                                             AF.Silu)
                    dbg_dump("hmid", h_sb, MC, TO)

                with tc.tile_pool(name="p7", bufs=1) as p7, \
                     tc.tile_pool(name="p7w", bufs=2) as p7w, \
                     tc.tile_pool(name="p7s", bufs=2) as p7s:
                    ym = p7.tile([P, KC, TO], F32, tag="ym")
                    psn = ps_long.tile([P, TO], F32, tag="av")
                    for oc in range(KC):
                        wt = p7w.tile([P, MC, P], MMDT, tag="w2")
                        dma(wt[:], w2_r[:, :, oc * P:(oc + 1) * P])
                        psA = ps_acc.tile([P, TO], F32, tag="acc")
                        for kc in range(MC):
                            nc.tensor.matmul(psA[:], wt[:, kc, :], h_sb[:, kc, :],
                                             start=(kc == 0), stop=(kc == MC - 1))
                        nc.scalar.copy(ym[:, oc, :], psA[:])
                        sqm = p7s.tile([P, TO], BF16, tag="sqm")
                        nc.vector.tensor_mul(sqm[:], ym[:, oc, :], ym[:, oc, :])
                        nc.tensor.matmul(psn[:], ones_c[:], sqm[:],
                                         start=(oc == 0), stop=(oc == KC - 1))
                    msc2 = pixel_scale(p7s, psn, HID, epsm, "msc2",
                                       lnbias=ln_mg)
                    out_t = p7.tile([P, KC, TO], F32, tag="out")
                    for oc in range(KC):
                        nc.vector.tensor_scalar_mul(out_t[:, oc, :],
                                                    xc1[:, oc, :], c1)
                        tmp = p7s.tile([P, TO], F32, tag="t7")
                        nc.vector.tensor_mul(tmp[:], ym[:, oc, :], msc2[:])
                        nc.vector.tensor_add(out_t[:, oc, :], out_t[:, oc, :],
                                             tmp[:])
                        dma(y_r[:, oc, :], out_t[:, oc, :])
    nc.compile()
    return nc


def kernel(x, c, w_cond, w_qkv, w_out, w_mlp1, w_mlp2, attn_gain, mlp_gain):
    import ml_dtypes
    from concourse.bass_utils import run_bass_kernel_spmd

    (wn_qkv, wn_out, wn_mlp1, wn_mlp2, g1, shift, ag_c, mg_c, c1) = _host_prep(
        x, c, w_cond, w_qkv, w_out, w_mlp1, w_mlp2, attn_gain, mlp_gain)

    wdt = np.float32 if _MMDT_NAME == "f32r" else ml_dtypes.bfloat16
    wn_qkv = np.ascontiguousarray(wn_qkv.astype(wdt))
    wn_out = np.ascontiguousarray(wn_out.astype(wdt))
    wn_mlp1 = np.ascontiguousarray(wn_mlp1.astype(wdt))
    wn_mlp2 = np.ascontiguousarray(wn_mlp2.astype(wdt))
    ones_c = np.ones((P, P), dtype=ml_dtypes.bfloat16)
    bd_c = np.zeros((P, P), dtype=np.float32)
    bd_c[0:DH, 0:DH] = 1.0
    bd_c[DH:P, DH:P] = 1.0
    bd_c = bd_c.astype(ml_dtypes.bfloat16)

    x32 = np.asarray(x, np.float32)
    g1_32 = g1.astype(np.float32)
    sh_32 = shift.astype(np.float32)

    in_maps = []
    for core in range(NCORES):
        b, half = core // 2, core % 2
        xb = x32[b]                                    # [N, HID]
        own = np.ascontiguousarray(xb[half * TO:(half + 1) * TO].T)
        oth = np.ascontiguousarray(xb[(1 - half) * TO:(2 - half) * TO].T)
        gs = np.empty((P, KC, 2), np.float32)
        gs[:, :, 0] = g1_32[b].reshape(KC, P).T
        gs[:, :, 1] = sh_32[b].reshape(KC, P).T
        in_maps.append({
            "x_own": own, "x_oth": oth, "gs": gs,
            "w_qkv": wn_qkv, "w_out": wn_out,
            "w_mlp1": wn_mlp1, "w_mlp2": wn_mlp2,
            "ones_c": ones_c, "bd_c": bd_c,
        })

    nc = _build_nc(ag_c, mg_c, c1)
    res = run_bass_kernel_spmd(nc, in_maps, core_ids=list(range(NCORES)),
                               trace=_TRACE)
    if _TRACE and res.exec_time_ns is not None:
        print(f"HW exec time: {res.exec_time_ns} ns")

    out = np.empty((B, N, HID), np.float32)
    for core in range(NCORES):
        b, half = core // 2, core % 2
        out[b, half * TO:(half + 1) * TO, :] = res.results[core]["y"].T
    kernel._last_results = res
    return out
